# revision 19
# baseline (speedup 1.0000x reference)
"""Trainium2 Bass kernel for nn_MiM_v2 (Mamba-style selective scan).

Sharding: pure data-parallel over batch B=16 across 8 NeuronCores
(2 batches per core, weights replicated, no collectives).

v5: software-pipelined emission. Engine queues are in-order, so chunk
c+1's front end (in_proj/conv/silu/x_proj/rmsnorm/B,C broadcast) is
woven between chunk c's per-d-tile back end (dt_proj/softplus/deltaA/
scan/C-mult/PE reduce) to keep the Vector engine fed across chunk
boundaries. Scan runs in place (hs overwrites bx). The 16 per-n scans
are fused into one instruction per d-tile by zeroing dA at n-seams.
in/dt/x/out projections all bf16; n-reduction + D_skip on PE.
"""

import sys

if "/opt/trn_rl_repo" not in sys.path:
    sys.path.insert(0, "/opt/trn_rl_repo")

import numpy as np
import ml_dtypes

import concourse.bass as bass
import concourse.mybir as mybir
import concourse.tile as tile
from concourse import bacc

# ---------------------------------------------------------------- constants
B, L, DM = 16, 1024, 512
DIN, DT, N, K = 2 * DM, 32, 16, 3
NCORES = 8
BPC = B // NCORES          # batches per core
T = BPC * L                # tokens per core
TC = 512                   # token chunk
NCH = T // TC              # chunks per core
CPB = L // TC              # chunks per batch
NDT = DIN // 128           # d-inner tiles
NKT = DM // 128            # k tiles for in_proj
NSPL = 10                  # n-streams whose bx/C mults run on DVE (rest gpsimd)

F32 = mybir.dt.float32
F32R = mybir.dt.float32r
BF16 = mybir.dt.bfloat16
AF = mybir.ActivationFunctionType
ALU = mybir.AluOpType

# after back-end d-tile d of chunk c, emit these front-end d-tiles of c+1
FE_SCHED = {0: (0, 1), 1: (2,), 2: (3,), 3: (4,), 4: (5,), 5: (6, 7)}
FE_STAGEC_AFTER = 5


# ---------------------------------------------------------------- host prep
def host_weights(inp):
    """Precompute transposed/reorganized weights (numpy, shared by all cores)."""
    f = lambda x: np.ascontiguousarray(np.asarray(x, np.float32))
    bf = lambda x: np.ascontiguousarray(
        np.asarray(x, np.float32).astype(ml_dtypes.bfloat16))
    w = {}
    win_T = np.asarray(inp["in_w"], np.float32).T      # (DM, DIN)
    cw = np.asarray(inp["conv_w"], np.float32)[:, 0, :]  # (DIN, 3)
    b_in = np.asarray(inp["in_b"], np.float32)         # (DIN,)
    conv_b = np.asarray(inp["conv_b"], np.float32)     # (DIN,)
    # conv folded into in_proj: tap k scales column d of in_w.T
    w["w_in_aug"] = bf(np.stack([win_T * cw[None, :, k] for k in range(3)]))
    # folded bias: b_in * sum(w_k) + conv_b  (exact for t >= 2 and for
    # chunks with real left context)
    w["b_fold"] = bf((b_in * cw.sum(1) + conv_b)[None, :])
    # negative boundary correction for t=0,1 of each sequence, where the
    # causal zero-pad is in h-space (bias must not leak into the pad)
    ncorr = np.zeros((2, DIN), np.float32)
    ncorr[0] = -(cw[:, 0] + cw[:, 1]) * b_in
    ncorr[1] = -cw[:, 0] * b_in
    w["ncorr"] = bf(ncorr.reshape(1, 2 * DIN))
    onehot = np.zeros((2, TC), np.float32)
    onehot[0, 0] = 1.0
    onehot[1, 1] = 1.0
    w["onehot"] = bf(onehot.reshape(1, 2 * TC))
    w["w_x_T"] = bf(inp["xproj_w"].T)                  # (DIN, DT+2N) bf16
    w["w_dt_T"] = bf(inp["dt_w"].T)                    # (DT, DIN) bf16
    w["w_out_T"] = bf(inp["out_w"].T)                  # (DIN, DM) bf16
    w["A_neg"] = f(-np.exp(np.asarray(inp["A_log"], np.float64)))  # (DIN, N)
    w["b_dt"] = bf(inp["dt_b"][None, :])               # (1, DIN) bf16
    w["b_out"] = bf(inp["out_b"][None, :])             # (1, DM) bf16
    # block-diag D_skip pieces, one (128,128) diag per d-tile, bf16
    D = np.asarray(inp["D_skip"], np.float32)
    ddiag = np.zeros((NDT, 128, 128), np.float32)
    for k in range(NDT):
        ddiag[k] = np.diag(D[k * 128:(k + 1) * 128])
    w["ddiag"] = bf(ddiag)
    w["eye"] = bf(np.eye(128, dtype=np.float32))
    w["lnw"] = f(np.concatenate(
        [inp["dtln_w"], inp["Bln_w"], inp["Cln_w"]])[:, None])  # (64, 1)
    m_ms = np.zeros((DT + 2 * N, 3), np.float32)
    m_ms[:DT, 0] = 1.0 / DT
    m_ms[DT:DT + N, 1] = 1.0 / N
    m_ms[DT + N:, 2] = 1.0 / N
    w["m_ms"] = bf(m_ms)
    e_bc = np.zeros((3, DT + 2 * N), np.float32)
    e_bc[0, :DT] = 1.0
    e_bc[1, DT:DT + N] = 1.0
    e_bc[2, DT + N:] = 1.0
    w["e_bc"] = bf(e_bc)
    w["ones_bf"] = bf(np.ones((1, TC), np.float32))
    return w


def host_x_shard(x, core):
    """x (B, L, DM) -> per-core transposed bf16 shard (BPC, DM, L)."""
    xs = np.asarray(x, np.float32)[core * BPC:(core + 1) * BPC]
    return np.ascontiguousarray(
        xs.transpose(0, 2, 1).astype(ml_dtypes.bfloat16))


# ---------------------------------------------------------------- IO decl
def declare_ios(nc):
    def d(name, shape=None, dt=F32):
        return nc.dram_tensor(name, list(shape), dt,
                              kind="ExternalInput").ap()
    ins = {
        "xT": d("xT", dt=BF16, shape=(BPC, DM, L)),
        "w_in_aug": d("w_in_aug", dt=BF16, shape=(3, DM, DIN)),
        "b_fold": d("b_fold", dt=BF16, shape=(1, DIN)),
        "ncorr": d("ncorr", dt=BF16, shape=(1, 2 * DIN)),
        "onehot": d("onehot", dt=BF16, shape=(1, 2 * TC)),
        "w_x_T": d("w_x_T", dt=BF16, shape=(DIN, DT + 2 * N)),
        "w_dt_T": d("w_dt_T", dt=BF16, shape=(DT, DIN)),
        "w_out_T": d("w_out_T", dt=BF16, shape=(DIN, DM)),
        "A_neg": d("A_neg", (DIN, N)),
        "b_dt": d("b_dt", dt=BF16, shape=(1, DIN)),
        "b_out": d("b_out", dt=BF16, shape=(1, DM)),
        "ddiag": d("ddiag", dt=BF16, shape=(NDT, 128, 128)),
        "eye": d("eye", dt=BF16, shape=(128, 128)),
        "lnw": d("lnw", (DT + 2 * N, 1)),
        "m_ms": d("m_ms", dt=BF16, shape=(DT + 2 * N, 3)),
        "e_bc": d("e_bc", dt=BF16, shape=(3, DT + 2 * N)),
        "ones_bf": d("ones_bf", dt=BF16, shape=(1, TC)),
    }
    outs = {
        "y_out": nc.dram_tensor("y_out", [BPC, L, DM], F32,
                                kind="ExternalOutput").ap(),
    }
    return ins, outs


# ---------------------------------------------------------------- kernel body
def emit(tc_ctx, outs, ins):
    from contextlib import ExitStack
    tc = tc_ctx
    nc = tc.nc
    G = DT + 2 * N  # 64

    st = ExitStack()
    pool = lambda **kw: st.enter_context(tc.tile_pool(**kw))
    cpool = pool(name="consts", bufs=1)
    xpool = pool(name="xck", bufs=1)
    hpool = pool(name="h", bufs=2)
    trpool = pool(name="transient", bufs=2)
    spool = pool(name="smalls", bufs=1)
    dnpool = pool(name="dn", bufs=2)
    dApool = pool(name="dA", bufs=2)
    bxpool = pool(name="bx", bufs=2)
    bcpool = pool(name="bcb", bufs=2)
    bcpoolC = pool(name="bcc", bufs=1)
    ypool = pool(name="y", bufs=1)
    opool = pool(name="osb", bufs=1)
    pp_h = pool(name="ph", bufs=2, space="PSUM")
    pp_misc = pool(name="pmisc", bufs=2, space="PSUM")
    pp_y = pool(name="py", bufs=2, space="PSUM")
    pp_o = pool(name="po", bufs=2, space="PSUM")

    dma = nc.sync.dma_start

    # ---- persistent constants -------------------------------------------
    def const_tile(name, shape=None, src=None, dt=F32):
        t = cpool.tile(list(shape), dt, tag=name)
        if src.dtype != dt and mybir.dt.size(src.dtype) == mybir.dt.size(dt):
            src = src.bitcast(dt)
        dma(t[:], src)
        return t

    # critical consts for chunk-0 front end load first so the pipeline
    # starts immediately; the rest stream in behind the first in_projs.
    in_waug = [[const_tile(f"in_wA{k}_{kt}", (128, DIN),
                           ins["w_in_aug"][k, kt * 128:(kt + 1) * 128, :],
                           dt=BF16)
                for kt in range(NKT)] for k in range(3)]
    b_fold = const_tile("b_fold", dt=BF16, shape=(1, DIN),
                        src=ins["b_fold"][:, :])
    ncorr = const_tile("ncorr", (1, 2 * DIN), ins["ncorr"][:, :], dt=BF16)
    onehot = const_tile("onehot", (1, 2 * TC), ins["onehot"][:, :], dt=BF16)
    ones_bf = const_tile("ones_bf", (1, TC), ins["ones_bf"][:, :], dt=BF16)
    eps = cpool.tile([128, 1], F32, tag="eps")
    nc.vector.memset(eps[:], 1e-5)

    xproj_wT, out_wT, A_sb, ddiag = [], [], [], []
    dfr = {}

    def load_deferred_consts():
        xproj_wT.extend(const_tile(f"xp_wT{k}", (128, G),
                                   ins["w_x_T"][k * 128:(k + 1) * 128, :],
                                   dt=BF16) for k in range(NDT))
        dfr["dt_wT"] = const_tile("dt_wT", (DT, DIN), ins["w_dt_T"][:, :],
                                  dt=BF16)
        A_sb.extend(const_tile(f"A{k}", (128, N),
                               ins["A_neg"][k * 128:(k + 1) * 128, :])
                    for k in range(NDT))
        dfr["eye"] = const_tile("eye", (128, 128), ins["eye"][:, :], dt=BF16)
        dfr["b_dt"] = const_tile("b_dt", dt=BF16, shape=(1, DIN),
                                 src=ins["b_dt"][:, :])
        dfr["lnw"] = const_tile("lnw", (G, 1), ins["lnw"][:, :])
        dfr["m_ms"] = const_tile("m_ms", (G, 3), ins["m_ms"][:, :], dt=BF16)
        dfr["e_bc"] = const_tile("e_bc", (3, G), ins["e_bc"][:, :], dt=BF16)
        out_wT.extend(const_tile(f"out_wT{k}", (128, DM),
                                 ins["w_out_T"][k * 128:(k + 1) * 128, :],
                                 dt=BF16) for k in range(NDT))
        ddiag.extend(const_tile(f"dd{k}", (128, 128), ins["ddiag"][k],
                                dt=BF16) for k in range(NDT))
        dfr["b_out"] = const_tile("b_out", dt=BF16, shape=(1, DM),
                                  src=ins["b_out"][:, :])

    # persistent cross-chunk state
    state = cpool.tile([128, NDT * N], F32, tag="state")      # scan carries

    # DRAM bounce buffer for the B/C broadcast
    bc_dram = nc.dram_tensor("bc_scratch", [NCH, 2 * N, TC], BF16).ap()

    # per-chunk live objects for the pipelined emission
    live = [dict() for _ in range(NCH)]

    # ---------------- front end ------------------------------------------
    def fe_prologue(ch):
        bb, cb = divmod(ch, CPB)
        xck = []
        for kt in range(NKT):
            # 2 extra leading columns of left context for the fused conv
            t = xpool.tile([128, TC + 2], BF16, tag=f"x{kt}")
            if cb == 0:
                nc.vector.memset(t[:, 0:2], 0.0)
                dma(t[:, 2:TC + 2], ins["xT"][bb, kt * 128:(kt + 1) * 128,
                                              0:TC])
            else:
                dma(t[:], ins["xT"][bb, kt * 128:(kt + 1) * 128,
                                    cb * TC - 2:(cb + 1) * TC])
            xck.append(t)
        live[ch]["xck"] = xck
        live[ch]["h"] = [None] * NDT

    def fe_dtile(ch, dt):
        bb, cb = divmod(ch, CPB)
        xck = live[ch]["xck"]
        ph = pp_h.tile([128, TC], F32, tag="ph")
        # in_proj and causal conv fused: tap k uses x shifted by 2-k
        for k in range(3):
            for kt in range(NKT):
                nc.tensor.matmul(
                    ph[:], in_waug[k][kt][:, dt * 128:(dt + 1) * 128],
                    xck[kt][:, k:k + TC], start=(k == 0 and kt == 0),
                    stop=False)
        if cb == 0:
            # cancel the bias leaked into the h-space zero padding
            for r in range(2):
                nc.tensor.matmul(
                    ph[:],
                    ncorr[0:1, r * DIN + dt * 128:r * DIN + (dt + 1) * 128],
                    onehot[0:1, r * TC:(r + 1) * TC], start=False,
                    stop=False)
        nc.tensor.matmul(
            ph[:], b_fold[0:1, dt * 128:(dt + 1) * 128],
            ones_bf[0:1, 0:TC], start=False, stop=True)
        h_t = hpool.tile([128, TC], BF16, tag=f"h{dt}")
        nc.scalar.activation(h_t[:], ph[:], AF.Silu)
        live[ch]["h"][dt] = h_t

    def fe_stagec(ch):
        h_list = live[ch]["h"]
        pdbc = pp_misc.tile([G, TC], F32, tag="pmisc")
        for kt in range(NDT):
            nc.tensor.matmul(pdbc[:], xproj_wT[kt][:], h_list[kt][:],
                             start=(kt == 0), stop=(kt == NDT - 1))
        dbc_sb = spool.tile([G, TC], F32, tag="dbc")
        nc.scalar.copy(dbc_sb[:], pdbc[:])
        sq = spool.tile([G, TC], BF16, tag="sq")
        nc.scalar.activation(sq[:], pdbc[:], AF.Square)
        pms = pp_misc.tile([3, TC], F32, tag="pmisc")
        nc.tensor.matmul(pms[:], dfr["m_ms"][:], sq[:], start=True, stop=True)
        lnm = spool.tile([3, TC], F32, tag="lnm")
        nc.scalar.activation(lnm[:], pms[:], AF.Ln, bias=eps[0:3, :])
        rin = spool.tile([3, TC], BF16, tag="rin")
        nc.scalar.activation(rin[:], lnm[:], AF.Exp, scale=-0.5)
        pr = pp_misc.tile([G, TC], F32, tag="pmisc")
        nc.tensor.matmul(pr[:], dfr["e_bc"][:], rin[:], start=True, stop=True)
        delta_n = dnpool.tile([DT, TC], BF16, tag="dn")
        nc.vector.scalar_tensor_tensor(
            delta_n[:], dbc_sb[0:DT, :], dfr["lnw"][0:DT, :], pr[0:DT, :],
            op0=ALU.mult, op1=ALU.mult)
        bc_n = spool.tile([2 * N, TC], BF16, tag="bcn")
        nc.vector.scalar_tensor_tensor(
            bc_n[:], dbc_sb[DT:G, :], dfr["lnw"][DT:G, :], pr[DT:G, :],
            op0=ALU.mult, op1=ALU.mult)

        # bounce B/C rows through DRAM to broadcast across 128 partitions
        dma(bc_dram[ch], bc_n[:])
        bcbB = bcpool.tile([128, N * TC], BF16, tag="bcb")
        nc.sync.dma_start(
            bcbB[:].rearrange("p (j t) -> p j t", j=N),
            bc_dram[ch, 0:N].unsqueeze(0).broadcast_to((128, N, TC)))
        bcbC = bcpoolC.tile([128, N * TC], BF16, tag="bcc")
        nc.sync.dma_start(
            bcbC[:].rearrange("p (j t) -> p j t", j=N),
            bc_dram[ch, N:2 * N].unsqueeze(0).broadcast_to((128, N, TC)))
        live[ch]["bcbB"] = bcbB
        live[ch]["bcbC"] = bcbC
        live[ch]["dn"] = delta_n

    # ---------------- back end -------------------------------------------
    def be_pre(ch, dt):
        """dt_proj -> softplus -> u, deltaA exps, bx build + carry fixup.
        Emitted one d-tile ahead so PE/ACT results are ready when the
        Vector engine reaches this d-tile's scan."""
        bb, cb = divmod(ch, CPB)
        h_t = live[ch]["h"][dt]
        delta_n = live[ch]["dn"]
        bcb_B3 = live[ch]["bcbB"][:].rearrange("p (n t) -> p n t", n=N)

        pd = pp_h.tile([128, TC], F32, tag="ph")
        nc.tensor.matmul(pd[:], dfr["dt_wT"][:, dt * 128:(dt + 1) * 128],
                         delta_n[:], start=True, stop=False)
        nc.tensor.matmul(pd[:], dfr["b_dt"][0:1, dt * 128:(dt + 1) * 128],
                         ones_bf[0:1, 0:TC], start=False, stop=True)
        esp = trpool.tile([128, TC], BF16, tag="esp")
        nc.scalar.activation(esp[:], pd[:], AF.Exp)
        delta_t = trpool.tile([128, TC], BF16, tag="delta")
        nc.scalar.activation(delta_t[:], esp[:], AF.Ln, bias=1.0)
        u_t = trpool.tile([128, TC], BF16, tag="u")
        nc.gpsimd.tensor_mul(u_t[:], delta_t[:], h_t[:])

        # deltaA = exp(A_n * delta), bf16, one [128, N*TC] tile
        dA = dApool.tile([128, N * TC], BF16, tag="dA")
        for n in range(N):
            nc.scalar.activation(
                dA[:, n * TC:(n + 1) * TC], delta_t[:], AF.Exp,
                scale=A_sb[dt][:, n:n + 1])

        # bx = u * B (broadcast u over n), split between gpsimd and DVE
        bx = bxpool.tile([128, N * TC], BF16, tag="bx")
        bx3 = bx[:].rearrange("p (n t) -> p n t", n=N)
        u3 = u_t[:].unsqueeze(1).broadcast_to((128, N, TC))
        nc.gpsimd.tensor_mul(bx3[:, NSPL:N, :], u3[:, NSPL:N, :],
                             bcb_B3[:, NSPL:N, :])
        nc.vector.tensor_mul(bx3[:, 0:NSPL, :], u3[:, 0:NSPL, :],
                             bcb_B3[:, 0:NSPL, :])

        dA3 = dA[:].rearrange("p (n t) -> p n t", n=N)
        # fold cross-chunk carry into bx[:, n*TC]
        if cb > 0:
            cfix = trpool.tile([128, N], F32, tag="cfix")
            nc.gpsimd.tensor_mul(cfix[:], dA3[:, :, 0],
                                 state[:, dt * N:(dt + 1) * N])
            nc.gpsimd.tensor_add(bx3[:, :, 0], bx3[:, :, 0], cfix[:])
        # zero dA at every n-seam so one long scan resets per n
        # (h_seam = 0*prev + bx_seam; carry already folded into bx)
        nc.gpsimd.memset(dA3[:, :, 0], 0.0)
        live[ch].setdefault("pre", {})[dt] = (dA, bx, bx3)

    def be_post(ch, dt):
        bb, cb = divmod(ch, CPB)
        h_t = live[ch]["h"][dt]
        dA, bx, bx3 = live[ch]["pre"].pop(dt)
        bcb_C3 = live[ch]["bcbC"][:].rearrange("p (n t) -> p n t", n=N)

        # the linear recurrence: one fused in-place scan (hs == bx)
        nc.vector.tensor_tensor_scan(
            bx[:, 0:N * TC], dA[:, 0:N * TC], bx[:, 0:N * TC],
            0.0, op0=ALU.mult, op1=ALU.add)
        if cb < CPB - 1:
            nc.gpsimd.tensor_copy(
                state[:, dt * N:(dt + 1) * N], bx3[:, :, TC - 1])

        # y_n = hs * C in place, then PE-reduce over n (+ D_skip * h)
        nc.gpsimd.tensor_mul(bx3[:, NSPL:N, :], bx3[:, NSPL:N, :],
                             bcb_C3[:, NSPL:N, :])
        nc.vector.tensor_mul(bx3[:, 0:NSPL, :], bx3[:, 0:NSPL, :],
                             bcb_C3[:, 0:NSPL, :])
        py = pp_y.tile([128, TC], F32, tag="py")
        for n in range(N):
            nc.tensor.matmul(py[:], dfr["eye"][:], bx[:, n * TC:(n + 1) * TC],
                             start=(n == 0), stop=False)
        nc.tensor.matmul(py[:], ddiag[dt][:], h_t[:],
                         start=False, stop=True)
        y_t = ypool.tile([128, TC], BF16, tag=f"y{dt}")
        nc.scalar.copy(y_t[:], py[:])
        live[ch].setdefault("y", [None] * NDT)[dt] = y_t

    def be_outproj(ch):
        bb, cb = divmod(ch, CPB)
        y_list = live[ch]["y"]
        for tt in range(TC // 128):
            po = pp_o.tile([128, DM], F32, tag="po")
            for dt in range(NDT):
                nc.tensor.matmul(
                    po[:], y_list[dt][:, tt * 128:(tt + 1) * 128],
                    out_wT[dt][:], start=(dt == 0), stop=False)
            nc.tensor.matmul(po[:], ones_bf[0:1, 0:128], dfr["b_out"][0:1, :],
                             start=False, stop=True)
            o_sb = opool.tile([128, DM], F32, tag="osb")
            nc.scalar.copy(o_sb[:], po[:])
            dma(outs["y_out"][bb, cb * TC + tt * 128:cb * TC + (tt + 1) * 128,
                              :], o_sb[:])
        live[ch].clear()

    # ---------------- pipelined emission ---------------------------------
    fe_prologue(0)
    for d in range(NDT):
        fe_dtile(0, d)
    load_deferred_consts()
    fe_stagec(0)
    be_pre(0, 0)
    for ch in range(NCH):
        nxt = ch + 1
        if nxt < NCH:
            fe_prologue(nxt)
        for d in range(NDT):
            # emit the NEXT d-tile's PE/ACT/bx work ahead of this scan
            if d + 1 < NDT:
                be_pre(ch, d + 1)
            elif nxt < NCH:
                be_pre(nxt, 0)
            be_post(ch, d)
            if nxt < NCH:
                for e in FE_SCHED.get(d, ()):
                    fe_dtile(nxt, e)
                if d == FE_STAGEC_AFTER:
                    fe_stagec(nxt)
        be_outproj(ch)

    st.close()


# ---------------------------------------------------------------- runner
_CACHE = {}


def _build_program():
    if "nc" in _CACHE:
        return _CACHE["nc"]
    nc = bacc.Bacc("TRN2", target_bir_lowering=False, debug=False,
                   num_devices=NCORES)
    ins, outs = declare_ios(nc)
    with tile.TileContext(nc) as t:
        emit(t, outs, ins)
    nc.compile()
    _CACHE["nc"] = nc
    return nc


LAST_RESULT = None


def kernel(**inputs) -> np.ndarray:
    global LAST_RESULT
    import os
    from concourse.bass_utils import run_bass_kernel_spmd

    nc = _build_program()
    w = host_weights(inputs)
    in_maps = []
    for c in range(NCORES):
        m = dict(w)
        m["xT"] = host_x_shard(inputs["x"], c)
        in_maps.append(m)
    trace = bool(os.environ.get("MIM_TRACE"))
    res = run_bass_kernel_spmd(nc, in_maps, list(range(NCORES)),
                               trace=trace)
    LAST_RESULT = res
    out = np.concatenate([res.results[c]["y_out"] for c in range(NCORES)],
                         axis=0)
    return out.astype(np.float32)


# revision 20
# speedup vs baseline: 1.2782x; 1.2782x over previous
"""Trainium2 Bass kernel for nn_MiM_v2 (Mamba-style selective scan).

Sharding: pure data-parallel over batch B=16 across 8 NeuronCores
(2 batches per core, weights replicated, no collectives).

v5: software-pipelined emission. Engine queues are in-order, so chunk
c+1's front end (in_proj/conv/silu/x_proj/rmsnorm/B,C broadcast) is
woven between chunk c's per-d-tile back end (dt_proj/softplus/deltaA/
scan/C-mult/PE reduce) to keep the Vector engine fed across chunk
boundaries. Scan runs in place (hs overwrites bx). The 16 per-n scans
are fused into one instruction per d-tile by zeroing dA at n-seams.
in/dt/x/out projections all bf16; n-reduction + D_skip on PE.
"""

import sys

if "/opt/trn_rl_repo" not in sys.path:
    sys.path.insert(0, "/opt/trn_rl_repo")

import numpy as np
import ml_dtypes

import concourse.bass as bass
import concourse.mybir as mybir
import concourse.tile as tile
from concourse import bacc

# ---------------------------------------------------------------- constants
B, L, DM = 16, 1024, 512
DIN, DT, N, K = 2 * DM, 32, 16, 3
NCORES = 8
BPC = B // NCORES          # batches per core
T = BPC * L                # tokens per core
TC = 512                   # token chunk
NCH = T // TC              # chunks per core
CPB = L // TC              # chunks per batch
NDT = DIN // 128           # d-inner tiles
NKT = DM // 128            # k tiles for in_proj
NSPL = 10                  # n-streams whose bx/C mults run on DVE (rest gpsimd)

F32 = mybir.dt.float32
F32R = mybir.dt.float32r
BF16 = mybir.dt.bfloat16
AF = mybir.ActivationFunctionType
ALU = mybir.AluOpType

# after back-end d-tile d of chunk c, emit these front-end d-tiles of c+1
FE_SCHED = {1: (0, 1, 2, 3), 4: (4, 5, 6, 7)}
FE_STAGEC_AFTER = 5


# ---------------------------------------------------------------- host prep
def host_weights(inp):
    """Precompute transposed/reorganized weights (numpy, shared by all cores)."""
    f = lambda x: np.ascontiguousarray(np.asarray(x, np.float32))
    bf = lambda x: np.ascontiguousarray(
        np.asarray(x, np.float32).astype(ml_dtypes.bfloat16))
    w = {}
    win_T = np.asarray(inp["in_w"], np.float32).T      # (DM, DIN)
    cw = np.asarray(inp["conv_w"], np.float32)[:, 0, :]  # (DIN, 3)
    b_in = np.asarray(inp["in_b"], np.float32)         # (DIN,)
    conv_b = np.asarray(inp["conv_b"], np.float32)     # (DIN,)
    # conv folded into in_proj: tap k scales column d of in_w.T
    w["w_in_aug"] = bf(np.stack([win_T * cw[None, :, k] for k in range(3)]))
    # folded bias: b_in * sum(w_k) + conv_b  (exact for t >= 2 and for
    # chunks with real left context)
    w["b_fold"] = bf((b_in * cw.sum(1) + conv_b)[None, :])
    # negative boundary correction for t=0,1 of each sequence, where the
    # causal zero-pad is in h-space (bias must not leak into the pad)
    ncorr = np.zeros((2, DIN), np.float32)
    ncorr[0] = -(cw[:, 0] + cw[:, 1]) * b_in
    ncorr[1] = -cw[:, 0] * b_in
    w["ncorr"] = bf(ncorr.reshape(1, 2 * DIN))
    onehot = np.zeros((2, TC), np.float32)
    onehot[0, 0] = 1.0
    onehot[1, 1] = 1.0
    w["onehot"] = bf(onehot.reshape(1, 2 * TC))
    w["w_x_T"] = bf(inp["xproj_w"].T)                  # (DIN, DT+2N) bf16
    w["w_dt_T"] = bf(inp["dt_w"].T)                    # (DT, DIN) bf16
    w["w_out_T"] = bf(inp["out_w"].T)                  # (DIN, DM) bf16
    w["A_neg"] = f(-np.exp(np.asarray(inp["A_log"], np.float64)))  # (DIN, N)
    w["b_dt"] = bf(inp["dt_b"][None, :])               # (1, DIN) bf16
    w["b_out"] = bf(inp["out_b"][None, :])             # (1, DM) bf16
    # block-diag D_skip pieces, one (128,128) diag per d-tile, bf16
    D = np.asarray(inp["D_skip"], np.float32)
    ddiag = np.zeros((NDT, 128, 128), np.float32)
    for k in range(NDT):
        ddiag[k] = np.diag(D[k * 128:(k + 1) * 128])
    w["ddiag"] = bf(ddiag)
    w["eye"] = bf(np.eye(128, dtype=np.float32))
    w["lnw"] = f(np.concatenate(
        [inp["dtln_w"], inp["Bln_w"], inp["Cln_w"]])[:, None])  # (64, 1)
    m_ms = np.zeros((DT + 2 * N, 3), np.float32)
    m_ms[:DT, 0] = 1.0 / DT
    m_ms[DT:DT + N, 1] = 1.0 / N
    m_ms[DT + N:, 2] = 1.0 / N
    w["m_ms"] = bf(m_ms)
    e_bc = np.zeros((3, DT + 2 * N), np.float32)
    e_bc[0, :DT] = 1.0
    e_bc[1, DT:DT + N] = 1.0
    e_bc[2, DT + N:] = 1.0
    w["e_bc"] = bf(e_bc)
    w["ones_bf"] = bf(np.ones((1, TC), np.float32))
    return w


def host_x_shard(x, core):
    """x (B, L, DM) -> per-core transposed bf16 shard (BPC, DM, L)."""
    xs = np.asarray(x, np.float32)[core * BPC:(core + 1) * BPC]
    return np.ascontiguousarray(
        xs.transpose(0, 2, 1).astype(ml_dtypes.bfloat16))


# ---------------------------------------------------------------- IO decl
def declare_ios(nc):
    def d(name, shape=None, dt=F32):
        return nc.dram_tensor(name, list(shape), dt,
                              kind="ExternalInput").ap()
    ins = {
        "xT": d("xT", dt=BF16, shape=(BPC, DM, L)),
        "w_in_aug": d("w_in_aug", dt=BF16, shape=(3, DM, DIN)),
        "b_fold": d("b_fold", dt=BF16, shape=(1, DIN)),
        "ncorr": d("ncorr", dt=BF16, shape=(1, 2 * DIN)),
        "onehot": d("onehot", dt=BF16, shape=(1, 2 * TC)),
        "w_x_T": d("w_x_T", dt=BF16, shape=(DIN, DT + 2 * N)),
        "w_dt_T": d("w_dt_T", dt=BF16, shape=(DT, DIN)),
        "w_out_T": d("w_out_T", dt=BF16, shape=(DIN, DM)),
        "A_neg": d("A_neg", (DIN, N)),
        "b_dt": d("b_dt", dt=BF16, shape=(1, DIN)),
        "b_out": d("b_out", dt=BF16, shape=(1, DM)),
        "ddiag": d("ddiag", dt=BF16, shape=(NDT, 128, 128)),
        "eye": d("eye", dt=BF16, shape=(128, 128)),
        "lnw": d("lnw", (DT + 2 * N, 1)),
        "m_ms": d("m_ms", dt=BF16, shape=(DT + 2 * N, 3)),
        "e_bc": d("e_bc", dt=BF16, shape=(3, DT + 2 * N)),
        "ones_bf": d("ones_bf", dt=BF16, shape=(1, TC)),
    }
    outs = {
        "y_out": nc.dram_tensor("y_out", [BPC, L, DM], F32,
                                kind="ExternalOutput").ap(),
    }
    return ins, outs


# ---------------------------------------------------------------- kernel body
def emit(tc_ctx, outs, ins):
    from contextlib import ExitStack
    tc = tc_ctx
    nc = tc.nc
    G = DT + 2 * N  # 64

    st = ExitStack()
    pool = lambda **kw: st.enter_context(tc.tile_pool(**kw))
    cpool = pool(name="consts", bufs=1)
    xpool = pool(name="xck", bufs=1)
    hpool = pool(name="h", bufs=2)
    trpool = pool(name="transient", bufs=2)
    spool = pool(name="smalls", bufs=1)
    dnpool = pool(name="dn", bufs=2)
    dApool = pool(name="dA", bufs=2)
    bxpool = pool(name="bx", bufs=2)
    bcpool = pool(name="bcb", bufs=2)
    bcpoolC = pool(name="bcc", bufs=1)
    ypool = pool(name="y", bufs=1)
    opool = pool(name="osb", bufs=1)
    pp_h = pool(name="ph", bufs=2, space="PSUM")
    pp_misc = pool(name="pmisc", bufs=2, space="PSUM")
    pp_y = pool(name="py", bufs=2, space="PSUM")
    pp_o = pool(name="po", bufs=2, space="PSUM")

    dma = nc.sync.dma_start

    # ---- persistent constants -------------------------------------------
    def const_tile(name, shape=None, src=None, dt=F32):
        t = cpool.tile(list(shape), dt, tag=name)
        if src.dtype != dt and mybir.dt.size(src.dtype) == mybir.dt.size(dt):
            src = src.bitcast(dt)
        dma(t[:], src)
        return t

    # critical consts for chunk-0 front end load first so the pipeline
    # starts immediately; the rest stream in behind the first in_projs.
    in_waug = [[const_tile(f"in_wA{k}_{kt}", (128, DIN),
                           ins["w_in_aug"][k, kt * 128:(kt + 1) * 128, :],
                           dt=BF16)
                for kt in range(NKT)] for k in range(3)]
    b_fold = const_tile("b_fold", dt=BF16, shape=(1, DIN),
                        src=ins["b_fold"][:, :])
    ncorr = const_tile("ncorr", (1, 2 * DIN), ins["ncorr"][:, :], dt=BF16)
    onehot = const_tile("onehot", (1, 2 * TC), ins["onehot"][:, :], dt=BF16)
    ones_bf = const_tile("ones_bf", (1, TC), ins["ones_bf"][:, :], dt=BF16)
    eps = cpool.tile([128, 1], F32, tag="eps")
    nc.vector.memset(eps[:], 1e-5)

    xproj_wT, out_wT, A_sb, ddiag = [], [], [], []
    dfr = {}

    def load_deferred_consts():
        xproj_wT.extend(const_tile(f"xp_wT{k}", (128, G),
                                   ins["w_x_T"][k * 128:(k + 1) * 128, :],
                                   dt=BF16) for k in range(NDT))
        dfr["dt_wT"] = const_tile("dt_wT", (DT, DIN), ins["w_dt_T"][:, :],
                                  dt=BF16)
        A_sb.extend(const_tile(f"A{k}", (128, N),
                               ins["A_neg"][k * 128:(k + 1) * 128, :])
                    for k in range(NDT))
        dfr["eye"] = const_tile("eye", (128, 128), ins["eye"][:, :], dt=BF16)
        dfr["b_dt"] = const_tile("b_dt", dt=BF16, shape=(1, DIN),
                                 src=ins["b_dt"][:, :])
        dfr["lnw"] = const_tile("lnw", (G, 1), ins["lnw"][:, :])
        dfr["m_ms"] = const_tile("m_ms", (G, 3), ins["m_ms"][:, :], dt=BF16)
        dfr["e_bc"] = const_tile("e_bc", (3, G), ins["e_bc"][:, :], dt=BF16)
        out_wT.extend(const_tile(f"out_wT{k}", (128, DM),
                                 ins["w_out_T"][k * 128:(k + 1) * 128, :],
                                 dt=BF16) for k in range(NDT))
        ddiag.extend(const_tile(f"dd{k}", (128, 128), ins["ddiag"][k],
                                dt=BF16) for k in range(NDT))
        dfr["b_out"] = const_tile("b_out", dt=BF16, shape=(1, DM),
                                  src=ins["b_out"][:, :])

    # persistent cross-chunk state
    state = cpool.tile([128, NDT * N], F32, tag="state")      # scan carries

    # DRAM bounce buffer for the B/C broadcast
    bc_dram = nc.dram_tensor("bc_scratch", [NCH, 2 * N, TC], BF16).ap()

    # per-chunk live objects for the pipelined emission
    live = [dict() for _ in range(NCH)]

    # ---------------- front end ------------------------------------------
    def fe_prologue(ch):
        bb, cb = divmod(ch, CPB)
        xck = []
        for kt in range(NKT):
            # 2 extra leading columns of left context for the fused conv
            t = xpool.tile([128, TC + 2], BF16, tag=f"x{kt}")
            if cb == 0:
                nc.vector.memset(t[:, 0:2], 0.0)
                dma(t[:, 2:TC + 2], ins["xT"][bb, kt * 128:(kt + 1) * 128,
                                              0:TC])
            else:
                dma(t[:], ins["xT"][bb, kt * 128:(kt + 1) * 128,
                                    cb * TC - 2:(cb + 1) * TC])
            xck.append(t)
        live[ch]["xck"] = xck
        live[ch]["h"] = [None] * NDT

    def fe_dtile(ch, dt):
        bb, cb = divmod(ch, CPB)
        xck = live[ch]["xck"]
        ph = pp_h.tile([128, TC], F32, tag="ph")
        # in_proj and causal conv fused: tap k uses x shifted by 2-k
        for k in range(3):
            for kt in range(NKT):
                nc.tensor.matmul(
                    ph[:], in_waug[k][kt][:, dt * 128:(dt + 1) * 128],
                    xck[kt][:, k:k + TC], start=(k == 0 and kt == 0),
                    stop=False)
        if cb == 0:
            # cancel the bias leaked into the h-space zero padding
            for r in range(2):
                nc.tensor.matmul(
                    ph[:],
                    ncorr[0:1, r * DIN + dt * 128:r * DIN + (dt + 1) * 128],
                    onehot[0:1, r * TC:(r + 1) * TC], start=False,
                    stop=False)
        nc.tensor.matmul(
            ph[:], b_fold[0:1, dt * 128:(dt + 1) * 128],
            ones_bf[0:1, 0:TC], start=False, stop=True)
        h_t = hpool.tile([128, TC], BF16, tag=f"h{dt}")
        nc.scalar.activation(h_t[:], ph[:], AF.Silu)
        live[ch]["h"][dt] = h_t

    def fe_stagec(ch):
        h_list = live[ch]["h"]
        pdbc = pp_misc.tile([G, TC], F32, tag="pmisc")
        for kt in range(NDT):
            nc.tensor.matmul(pdbc[:], xproj_wT[kt][:], h_list[kt][:],
                             start=(kt == 0), stop=(kt == NDT - 1))
        dbc_sb = spool.tile([G, TC], F32, tag="dbc")
        nc.scalar.copy(dbc_sb[:], pdbc[:])
        sq = spool.tile([G, TC], BF16, tag="sq")
        nc.scalar.activation(sq[:], pdbc[:], AF.Square)
        pms = pp_misc.tile([3, TC], F32, tag="pmisc")
        nc.tensor.matmul(pms[:], dfr["m_ms"][:], sq[:], start=True, stop=True)
        lnm = spool.tile([3, TC], F32, tag="lnm")
        nc.scalar.activation(lnm[:], pms[:], AF.Ln, bias=eps[0:3, :])
        rin = spool.tile([3, TC], BF16, tag="rin")
        nc.scalar.activation(rin[:], lnm[:], AF.Exp, scale=-0.5)
        pr = pp_misc.tile([G, TC], F32, tag="pmisc")
        nc.tensor.matmul(pr[:], dfr["e_bc"][:], rin[:], start=True, stop=True)
        delta_n = dnpool.tile([DT, TC], BF16, tag="dn")
        nc.vector.scalar_tensor_tensor(
            delta_n[:], dbc_sb[0:DT, :], dfr["lnw"][0:DT, :], pr[0:DT, :],
            op0=ALU.mult, op1=ALU.mult)
        bc_n = spool.tile([2 * N, TC], BF16, tag="bcn")
        nc.vector.scalar_tensor_tensor(
            bc_n[:], dbc_sb[DT:G, :], dfr["lnw"][DT:G, :], pr[DT:G, :],
            op0=ALU.mult, op1=ALU.mult)

        # bounce B/C rows through DRAM to broadcast across 128 partitions
        dma(bc_dram[ch], bc_n[:])
        bcbB = bcpool.tile([128, N * TC], BF16, tag="bcb")
        nc.sync.dma_start(
            bcbB[:].rearrange("p (j t) -> p j t", j=N),
            bc_dram[ch, 0:N].unsqueeze(0).broadcast_to((128, N, TC)))
        bcbC = bcpoolC.tile([128, N * TC], BF16, tag="bcc")
        nc.sync.dma_start(
            bcbC[:].rearrange("p (j t) -> p j t", j=N),
            bc_dram[ch, N:2 * N].unsqueeze(0).broadcast_to((128, N, TC)))
        live[ch]["bcbB"] = bcbB
        live[ch]["bcbC"] = bcbC
        live[ch]["dn"] = delta_n

    # ---------------- back end -------------------------------------------
    def be_pre(ch, dt):
        """dt_proj -> softplus -> u, deltaA exps, bx build + carry fixup.
        Emitted one d-tile ahead so PE/ACT results are ready when the
        Vector engine reaches this d-tile's scan."""
        bb, cb = divmod(ch, CPB)
        h_t = live[ch]["h"][dt]
        delta_n = live[ch]["dn"]
        bcb_B3 = live[ch]["bcbB"][:].rearrange("p (n t) -> p n t", n=N)

        pd = pp_h.tile([128, TC], F32, tag="ph")
        nc.tensor.matmul(pd[:], dfr["dt_wT"][:, dt * 128:(dt + 1) * 128],
                         delta_n[:], start=True, stop=False)
        nc.tensor.matmul(pd[:], dfr["b_dt"][0:1, dt * 128:(dt + 1) * 128],
                         ones_bf[0:1, 0:TC], start=False, stop=True)
        esp = trpool.tile([128, TC], BF16, tag="esp")
        nc.scalar.activation(esp[:], pd[:], AF.Exp)
        delta_t = trpool.tile([128, TC], BF16, tag="delta")
        nc.scalar.activation(delta_t[:], esp[:], AF.Ln, bias=1.0)
        u_t = trpool.tile([128, TC], BF16, tag="u")
        nc.gpsimd.tensor_mul(u_t[:], delta_t[:], h_t[:])

        # deltaA = exp(A_n * delta), bf16, one [128, N*TC] tile
        dA = dApool.tile([128, N * TC], BF16, tag="dA")
        for n in range(N):
            nc.scalar.activation(
                dA[:, n * TC:(n + 1) * TC], delta_t[:], AF.Exp,
                scale=A_sb[dt][:, n:n + 1])

        # bx = u * B (broadcast u over n), split between gpsimd and DVE
        bx = bxpool.tile([128, N * TC], BF16, tag="bx")
        bx3 = bx[:].rearrange("p (n t) -> p n t", n=N)
        u3 = u_t[:].unsqueeze(1).broadcast_to((128, N, TC))
        nc.vector.tensor_mul(bx3, u3, bcb_B3)

        dA3 = dA[:].rearrange("p (n t) -> p n t", n=N)
        # fold cross-chunk carry into bx[:, n*TC]
        if cb > 0:
            cfix = trpool.tile([128, N], F32, tag="cfix")
            nc.gpsimd.tensor_mul(cfix[:], dA3[:, :, 0],
                                 state[:, dt * N:(dt + 1) * N])
            nc.gpsimd.tensor_add(bx3[:, :, 0], bx3[:, :, 0], cfix[:])
        # zero dA at every n-seam so one long scan resets per n
        # (h_seam = 0*prev + bx_seam; carry already folded into bx)
        nc.gpsimd.memset(dA3[:, :, 0], 0.0)
        live[ch].setdefault("pre", {})[dt] = (dA, bx, bx3)

    def be_post(ch, dt):
        bb, cb = divmod(ch, CPB)
        h_t = live[ch]["h"][dt]
        dA, bx, bx3 = live[ch]["pre"].pop(dt)
        bcb_C3 = live[ch]["bcbC"][:].rearrange("p (n t) -> p n t", n=N)

        # the linear recurrence: one fused in-place scan (hs == bx)
        nc.vector.tensor_tensor_scan(
            bx[:, 0:N * TC], dA[:, 0:N * TC], bx[:, 0:N * TC],
            0.0, op0=ALU.mult, op1=ALU.add)
        if cb < CPB - 1:
            nc.gpsimd.tensor_copy(
                state[:, dt * N:(dt + 1) * N], bx3[:, :, TC - 1])

        # y_n = hs * C in place, then PE-reduce over n (+ D_skip * h)
        nc.vector.tensor_mul(bx3, bx3, bcb_C3)
        py = pp_y.tile([128, TC], F32, tag="py")
        for n in range(N):
            nc.tensor.matmul(py[:], dfr["eye"][:], bx[:, n * TC:(n + 1) * TC],
                             start=(n == 0), stop=False)
        nc.tensor.matmul(py[:], ddiag[dt][:], h_t[:],
                         start=False, stop=True)
        y_t = ypool.tile([128, TC], BF16, tag=f"y{dt}")
        nc.scalar.copy(y_t[:], py[:])
        live[ch].setdefault("y", [None] * NDT)[dt] = y_t

    def be_outproj(ch):
        bb, cb = divmod(ch, CPB)
        y_list = live[ch]["y"]
        for tt in range(TC // 128):
            po = pp_o.tile([128, DM], F32, tag="po")
            for dt in range(NDT):
                nc.tensor.matmul(
                    po[:], y_list[dt][:, tt * 128:(tt + 1) * 128],
                    out_wT[dt][:], start=(dt == 0), stop=False)
            nc.tensor.matmul(po[:], ones_bf[0:1, 0:128], dfr["b_out"][0:1, :],
                             start=False, stop=True)
            o_sb = opool.tile([128, DM], F32, tag="osb")
            nc.scalar.copy(o_sb[:], po[:])
            dma(outs["y_out"][bb, cb * TC + tt * 128:cb * TC + (tt + 1) * 128,
                              :], o_sb[:])
        live[ch].clear()

    # ---------------- pipelined emission ---------------------------------
    fe_prologue(0)
    for d in range(NDT):
        fe_dtile(0, d)
    load_deferred_consts()
    fe_stagec(0)
    be_pre(0, 0)
    for ch in range(NCH):
        nxt = ch + 1
        if nxt < NCH:
            fe_prologue(nxt)
        for d in range(NDT):
            # emit the NEXT d-tile's PE/ACT/bx work ahead of this scan
            if d + 1 < NDT:
                be_pre(ch, d + 1)
            elif nxt < NCH:
                be_pre(nxt, 0)
            be_post(ch, d)
            if nxt < NCH:
                for e in FE_SCHED.get(d, ()):
                    fe_dtile(nxt, e)
                if d == FE_STAGEC_AFTER:
                    fe_stagec(nxt)
        be_outproj(ch)

    st.close()


# ---------------------------------------------------------------- runner
_CACHE = {}


def _build_program():
    if "nc" in _CACHE:
        return _CACHE["nc"]
    nc = bacc.Bacc("TRN2", target_bir_lowering=False, debug=False,
                   num_devices=NCORES)
    ins, outs = declare_ios(nc)
    with tile.TileContext(nc) as t:
        emit(t, outs, ins)
    nc.compile()
    _CACHE["nc"] = nc
    return nc


LAST_RESULT = None


def kernel(**inputs) -> np.ndarray:
    global LAST_RESULT
    import os
    from concourse.bass_utils import run_bass_kernel_spmd

    nc = _build_program()
    w = host_weights(inputs)
    in_maps = []
    for c in range(NCORES):
        m = dict(w)
        m["xT"] = host_x_shard(inputs["x"], c)
        in_maps.append(m)
    trace = bool(os.environ.get("MIM_TRACE"))
    res = run_bass_kernel_spmd(nc, in_maps, list(range(NCORES)),
                               trace=trace)
    LAST_RESULT = res
    out = np.concatenate([res.results[c]["y_out"] for c in range(NCORES)],
                         axis=0)
    return out.astype(np.float32)


# revision 21
# speedup vs baseline: 1.2938x; 1.0122x over previous
"""Trainium2 Bass kernel for nn_MiM_v2 (Mamba-style selective scan).

Sharding: pure data-parallel over batch B=16 across 8 NeuronCores
(2 batches per core, weights replicated, no collectives).

v5: software-pipelined emission. Engine queues are in-order, so chunk
c+1's front end (in_proj/conv/silu/x_proj/rmsnorm/B,C broadcast) is
woven between chunk c's per-d-tile back end (dt_proj/softplus/deltaA/
scan/C-mult/PE reduce) to keep the Vector engine fed across chunk
boundaries. Scan runs in place (hs overwrites bx). The 16 per-n scans
are fused into one instruction per d-tile by zeroing dA at n-seams.
in/dt/x/out projections all bf16; n-reduction + D_skip on PE.
"""

import sys

if "/opt/trn_rl_repo" not in sys.path:
    sys.path.insert(0, "/opt/trn_rl_repo")

import numpy as np
import ml_dtypes

import concourse.bass as bass
import concourse.mybir as mybir
import concourse.tile as tile
from concourse import bacc

# ---------------------------------------------------------------- constants
B, L, DM = 16, 1024, 512
DIN, DT, N, K = 2 * DM, 32, 16, 3
NCORES = 8
BPC = B // NCORES          # batches per core
T = BPC * L                # tokens per core
TC = 512                   # token chunk
NCH = T // TC              # chunks per core
CPB = L // TC              # chunks per batch
NDT = DIN // 128           # d-inner tiles
NKT = DM // 128            # k tiles for in_proj
NSPL = 10                  # n-streams whose bx/C mults run on DVE (rest gpsimd)

F32 = mybir.dt.float32
F32R = mybir.dt.float32r
BF16 = mybir.dt.bfloat16
AF = mybir.ActivationFunctionType
ALU = mybir.AluOpType

# after back-end d-tile d of chunk c, emit these front-end d-tiles of c+1
FE_SCHED = {1: (0, 1, 2, 3), 4: (4, 5, 6, 7)}
FE_STAGEC_AFTER = 5


# ---------------------------------------------------------------- host prep
def host_weights(inp):
    """Precompute transposed/reorganized weights (numpy, shared by all cores)."""
    f = lambda x: np.ascontiguousarray(np.asarray(x, np.float32))
    bf = lambda x: np.ascontiguousarray(
        np.asarray(x, np.float32).astype(ml_dtypes.bfloat16))
    w = {}
    win_T = np.asarray(inp["in_w"], np.float32).T      # (DM, DIN)
    cw = np.asarray(inp["conv_w"], np.float32)[:, 0, :]  # (DIN, 3)
    b_in = np.asarray(inp["in_b"], np.float32)         # (DIN,)
    conv_b = np.asarray(inp["conv_b"], np.float32)     # (DIN,)
    # conv folded into in_proj: tap k scales column d of in_w.T
    w["w_in_aug"] = bf(np.stack([win_T * cw[None, :, k] for k in range(3)]))
    # folded bias: b_in * sum(w_k) + conv_b  (exact for t >= 2 and for
    # chunks with real left context)
    w["b_fold"] = bf((b_in * cw.sum(1) + conv_b)[None, :])
    # negative boundary correction for t=0,1 of each sequence, where the
    # causal zero-pad is in h-space (bias must not leak into the pad)
    ncorr = np.zeros((2, DIN), np.float32)
    ncorr[0] = -(cw[:, 0] + cw[:, 1]) * b_in
    ncorr[1] = -cw[:, 0] * b_in
    w["ncorr"] = bf(ncorr.reshape(1, 2 * DIN))
    onehot = np.zeros((2, TC), np.float32)
    onehot[0, 0] = 1.0
    onehot[1, 1] = 1.0
    w["onehot"] = bf(onehot.reshape(1, 2 * TC))
    w["w_x_T"] = bf(inp["xproj_w"].T)                  # (DIN, DT+2N) bf16
    w["w_dt_T"] = bf(inp["dt_w"].T)                    # (DT, DIN) bf16
    w["w_out_T"] = bf(inp["out_w"].T)                  # (DIN, DM) bf16
    w["A_neg"] = f(-np.exp(np.asarray(inp["A_log"], np.float64)))  # (DIN, N)
    w["b_dt"] = bf(inp["dt_b"][None, :])               # (1, DIN) bf16
    w["b_out"] = bf(inp["out_b"][None, :])             # (1, DM) bf16
    # block-diag D_skip pieces, one (128,128) diag per d-tile, bf16
    D = np.asarray(inp["D_skip"], np.float32)
    ddiag = np.zeros((NDT, 128, 128), np.float32)
    for k in range(NDT):
        ddiag[k] = np.diag(D[k * 128:(k + 1) * 128])
    w["ddiag"] = bf(ddiag)
    w["eye"] = bf(np.eye(128, dtype=np.float32))
    w["lnw"] = f(np.concatenate(
        [inp["dtln_w"], inp["Bln_w"], inp["Cln_w"]])[:, None])  # (64, 1)
    m_ms = np.zeros((DT + 2 * N, 3), np.float32)
    m_ms[:DT, 0] = 1.0 / DT
    m_ms[DT:DT + N, 1] = 1.0 / N
    m_ms[DT + N:, 2] = 1.0 / N
    w["m_ms"] = bf(m_ms)
    e_bc = np.zeros((3, DT + 2 * N), np.float32)
    e_bc[0, :DT] = 1.0
    e_bc[1, DT:DT + N] = 1.0
    e_bc[2, DT + N:] = 1.0
    w["e_bc"] = bf(e_bc)
    w["ones_bf"] = bf(np.ones((1, TC), np.float32))
    return w


def host_x_shard(x, core):
    """x (B, L, DM) -> per-core transposed bf16 shard (BPC, DM, L)."""
    xs = np.asarray(x, np.float32)[core * BPC:(core + 1) * BPC]
    return np.ascontiguousarray(
        xs.transpose(0, 2, 1).astype(ml_dtypes.bfloat16))


# ---------------------------------------------------------------- IO decl
def declare_ios(nc):
    def d(name, shape=None, dt=F32):
        return nc.dram_tensor(name, list(shape), dt,
                              kind="ExternalInput").ap()
    ins = {
        "xT": d("xT", dt=BF16, shape=(BPC, DM, L)),
        "w_in_aug": d("w_in_aug", dt=BF16, shape=(3, DM, DIN)),
        "b_fold": d("b_fold", dt=BF16, shape=(1, DIN)),
        "ncorr": d("ncorr", dt=BF16, shape=(1, 2 * DIN)),
        "onehot": d("onehot", dt=BF16, shape=(1, 2 * TC)),
        "w_x_T": d("w_x_T", dt=BF16, shape=(DIN, DT + 2 * N)),
        "w_dt_T": d("w_dt_T", dt=BF16, shape=(DT, DIN)),
        "w_out_T": d("w_out_T", dt=BF16, shape=(DIN, DM)),
        "A_neg": d("A_neg", (DIN, N)),
        "b_dt": d("b_dt", dt=BF16, shape=(1, DIN)),
        "b_out": d("b_out", dt=BF16, shape=(1, DM)),
        "ddiag": d("ddiag", dt=BF16, shape=(NDT, 128, 128)),
        "eye": d("eye", dt=BF16, shape=(128, 128)),
        "lnw": d("lnw", (DT + 2 * N, 1)),
        "m_ms": d("m_ms", dt=BF16, shape=(DT + 2 * N, 3)),
        "e_bc": d("e_bc", dt=BF16, shape=(3, DT + 2 * N)),
        "ones_bf": d("ones_bf", dt=BF16, shape=(1, TC)),
    }
    outs = {
        "y_out": nc.dram_tensor("y_out", [BPC, L, DM], F32,
                                kind="ExternalOutput").ap(),
    }
    return ins, outs


# ---------------------------------------------------------------- kernel body
def emit(tc_ctx, outs, ins):
    from contextlib import ExitStack
    tc = tc_ctx
    nc = tc.nc
    G = DT + 2 * N  # 64

    st = ExitStack()
    pool = lambda **kw: st.enter_context(tc.tile_pool(**kw))
    cpool = pool(name="consts", bufs=1)
    xpool = pool(name="xck", bufs=1)
    hpool = pool(name="h", bufs=2)
    trpool = pool(name="transient", bufs=2)
    spool = pool(name="smalls", bufs=1)
    dnpool = pool(name="dn", bufs=2)
    dApool = pool(name="dA", bufs=2)
    bxpool = pool(name="bx", bufs=2)
    bcpool = pool(name="bcb", bufs=2)
    bcpoolC = pool(name="bcc", bufs=1)
    ypool = pool(name="y", bufs=1)
    opool = pool(name="osb", bufs=1)
    pp_h = pool(name="ph", bufs=2, space="PSUM")
    pp_misc = pool(name="pmisc", bufs=2, space="PSUM")
    pp_y = pool(name="py", bufs=2, space="PSUM")
    pp_o = pool(name="po", bufs=2, space="PSUM")

    dma = nc.sync.dma_start

    # ---- persistent constants -------------------------------------------
    def const_tile(name, shape=None, src=None, dt=F32):
        t = cpool.tile(list(shape), dt, tag=name)
        if src.dtype != dt and mybir.dt.size(src.dtype) == mybir.dt.size(dt):
            src = src.bitcast(dt)
        dma(t[:], src)
        return t

    # critical consts for chunk-0 front end load first so the pipeline
    # starts immediately; the rest stream in behind the first in_projs.
    in_waug = [[const_tile(f"in_wA{k}_{kt}", (128, DIN),
                           ins["w_in_aug"][k, kt * 128:(kt + 1) * 128, :],
                           dt=BF16)
                for kt in range(NKT)] for k in range(3)]
    b_fold = const_tile("b_fold", dt=BF16, shape=(1, DIN),
                        src=ins["b_fold"][:, :])
    ncorr = const_tile("ncorr", (1, 2 * DIN), ins["ncorr"][:, :], dt=BF16)
    onehot = const_tile("onehot", (1, 2 * TC), ins["onehot"][:, :], dt=BF16)
    ones_bf = const_tile("ones_bf", (1, TC), ins["ones_bf"][:, :], dt=BF16)
    eps = cpool.tile([128, 1], F32, tag="eps")
    nc.vector.memset(eps[:], 1e-5)

    xproj_wT, out_wT, A_sb, ddiag = [], [], [], []
    dfr = {}

    def load_deferred_consts():
        xproj_wT.extend(const_tile(f"xp_wT{k}", (128, G),
                                   ins["w_x_T"][k * 128:(k + 1) * 128, :],
                                   dt=BF16) for k in range(NDT))
        dfr["dt_wT"] = const_tile("dt_wT", (DT, DIN), ins["w_dt_T"][:, :],
                                  dt=BF16)
        A_sb.extend(const_tile(f"A{k}", (128, N),
                               ins["A_neg"][k * 128:(k + 1) * 128, :])
                    for k in range(NDT))
        dfr["eye"] = const_tile("eye", (128, 128), ins["eye"][:, :], dt=BF16)
        dfr["b_dt"] = const_tile("b_dt", dt=BF16, shape=(1, DIN),
                                 src=ins["b_dt"][:, :])
        dfr["lnw"] = const_tile("lnw", (G, 1), ins["lnw"][:, :])
        dfr["m_ms"] = const_tile("m_ms", (G, 3), ins["m_ms"][:, :], dt=BF16)
        dfr["e_bc"] = const_tile("e_bc", (3, G), ins["e_bc"][:, :], dt=BF16)
        out_wT.extend(const_tile(f"out_wT{k}", (128, DM),
                                 ins["w_out_T"][k * 128:(k + 1) * 128, :],
                                 dt=BF16) for k in range(NDT))
        ddiag.extend(const_tile(f"dd{k}", (128, 128), ins["ddiag"][k],
                                dt=BF16) for k in range(NDT))
        dfr["b_out"] = const_tile("b_out", dt=BF16, shape=(1, DM),
                                  src=ins["b_out"][:, :])

    # persistent cross-chunk state
    state = cpool.tile([128, NDT * N], F32, tag="state")      # scan carries

    # DRAM bounce buffer for the B/C broadcast
    bc_dram = nc.dram_tensor("bc_scratch", [NCH, 2 * N, TC], BF16).ap()

    # per-chunk live objects for the pipelined emission
    live = [dict() for _ in range(NCH)]

    # ---------------- front end ------------------------------------------
    def fe_prologue(ch):
        bb, cb = divmod(ch, CPB)
        xck = []
        for kt in range(NKT):
            # 2 extra leading columns of left context for the fused conv
            t = xpool.tile([128, TC + 2], BF16, tag=f"x{kt}")
            if cb == 0:
                nc.vector.memset(t[:, 0:2], 0.0)
                dma(t[:, 2:TC + 2], ins["xT"][bb, kt * 128:(kt + 1) * 128,
                                              0:TC])
            else:
                dma(t[:], ins["xT"][bb, kt * 128:(kt + 1) * 128,
                                    cb * TC - 2:(cb + 1) * TC])
            xck.append(t)
        live[ch]["xck"] = xck
        live[ch]["h"] = [None] * NDT

    def fe_dtile(ch, dt):
        bb, cb = divmod(ch, CPB)
        xck = live[ch]["xck"]
        ph = pp_h.tile([128, TC], F32, tag="ph")
        # in_proj and causal conv fused: tap k uses x shifted by 2-k
        for k in range(3):
            for kt in range(NKT):
                nc.tensor.matmul(
                    ph[:], in_waug[k][kt][:, dt * 128:(dt + 1) * 128],
                    xck[kt][:, k:k + TC], start=(k == 0 and kt == 0),
                    stop=False)
        if cb == 0:
            # cancel the bias leaked into the h-space zero padding
            for r in range(2):
                nc.tensor.matmul(
                    ph[:],
                    ncorr[0:1, r * DIN + dt * 128:r * DIN + (dt + 1) * 128],
                    onehot[0:1, r * TC:(r + 1) * TC], start=False,
                    stop=False)
        nc.tensor.matmul(
            ph[:], b_fold[0:1, dt * 128:(dt + 1) * 128],
            ones_bf[0:1, 0:TC], start=False, stop=True)
        h_t = hpool.tile([128, TC], BF16, tag=f"h{dt}")
        nc.scalar.activation(h_t[:], ph[:], AF.Silu)
        live[ch]["h"][dt] = h_t

    def fe_stagec(ch):
        h_list = live[ch]["h"]
        pdbc = pp_misc.tile([G, TC], F32, tag="pmisc")
        for kt in range(NDT):
            nc.tensor.matmul(pdbc[:], xproj_wT[kt][:], h_list[kt][:],
                             start=(kt == 0), stop=(kt == NDT - 1))
        dbc_sb = spool.tile([G, TC], F32, tag="dbc")
        nc.scalar.copy(dbc_sb[:], pdbc[:])
        sq = spool.tile([G, TC], BF16, tag="sq")
        nc.scalar.activation(sq[:], pdbc[:], AF.Square)
        pms = pp_misc.tile([3, TC], F32, tag="pmisc")
        nc.tensor.matmul(pms[:], dfr["m_ms"][:], sq[:], start=True, stop=True)
        lnm = spool.tile([3, TC], F32, tag="lnm")
        nc.scalar.activation(lnm[:], pms[:], AF.Ln, bias=eps[0:3, :])
        rin = spool.tile([3, TC], BF16, tag="rin")
        nc.scalar.activation(rin[:], lnm[:], AF.Exp, scale=-0.5)
        pr = pp_misc.tile([G, TC], F32, tag="pmisc")
        nc.tensor.matmul(pr[:], dfr["e_bc"][:], rin[:], start=True, stop=True)
        delta_n = dnpool.tile([DT, TC], BF16, tag="dn")
        nc.vector.scalar_tensor_tensor(
            delta_n[:], dbc_sb[0:DT, :], dfr["lnw"][0:DT, :], pr[0:DT, :],
            op0=ALU.mult, op1=ALU.mult)
        bc_n = spool.tile([2 * N, TC], BF16, tag="bcn")
        nc.vector.scalar_tensor_tensor(
            bc_n[:], dbc_sb[DT:G, :], dfr["lnw"][DT:G, :], pr[DT:G, :],
            op0=ALU.mult, op1=ALU.mult)

        # bounce B/C rows through DRAM to broadcast across 128 partitions
        dma(bc_dram[ch], bc_n[:])
        bcbB = bcpool.tile([128, N * TC], BF16, tag="bcb")
        nc.sync.dma_start(
            bcbB[:].rearrange("p (j t) -> p j t", j=N),
            bc_dram[ch, 0:N].unsqueeze(0).broadcast_to((128, N, TC)))
        bcbC = bcpoolC.tile([128, N * TC], BF16, tag="bcc")
        nc.sync.dma_start(
            bcbC[:].rearrange("p (j t) -> p j t", j=N),
            bc_dram[ch, N:2 * N].unsqueeze(0).broadcast_to((128, N, TC)))
        live[ch]["bcbB"] = bcbB
        live[ch]["bcbC"] = bcbC
        live[ch]["dn"] = delta_n

    # ---------------- back end -------------------------------------------
    def be_pre(ch, dt):
        """dt_proj -> softplus -> u, deltaA exps, bx build + carry fixup.
        Emitted one d-tile ahead so PE/ACT results are ready when the
        Vector engine reaches this d-tile's scan."""
        bb, cb = divmod(ch, CPB)
        h_t = live[ch]["h"][dt]
        delta_n = live[ch]["dn"]
        bcb_B3 = live[ch]["bcbB"][:].rearrange("p (n t) -> p n t", n=N)

        pd = pp_h.tile([128, TC], F32, tag="ph")
        nc.tensor.matmul(pd[:], dfr["dt_wT"][:, dt * 128:(dt + 1) * 128],
                         delta_n[:], start=True, stop=False)
        nc.tensor.matmul(pd[:], dfr["b_dt"][0:1, dt * 128:(dt + 1) * 128],
                         ones_bf[0:1, 0:TC], start=False, stop=True)
        esp = trpool.tile([128, TC], BF16, tag="esp")
        nc.scalar.activation(esp[:], pd[:], AF.Exp)
        delta_t = trpool.tile([128, TC], BF16, tag="delta")
        nc.scalar.activation(delta_t[:], esp[:], AF.Ln, bias=1.0)
        u_t = trpool.tile([128, TC], BF16, tag="u")
        nc.vector.tensor_mul(u_t[:], delta_t[:], h_t[:])

        # deltaA = exp(A_n * delta), bf16, one [128, N*TC] tile
        dA = dApool.tile([128, N * TC], BF16, tag="dA")
        for n in range(N):
            nc.scalar.activation(
                dA[:, n * TC:(n + 1) * TC], delta_t[:], AF.Exp,
                scale=A_sb[dt][:, n:n + 1])

        # bx = u * B (broadcast u over n), split between gpsimd and DVE
        bx = bxpool.tile([128, N * TC], BF16, tag="bx")
        bx3 = bx[:].rearrange("p (n t) -> p n t", n=N)
        u3 = u_t[:].unsqueeze(1).broadcast_to((128, N, TC))
        nc.vector.tensor_mul(bx3, u3, bcb_B3)

        dA3 = dA[:].rearrange("p (n t) -> p n t", n=N)
        # fold cross-chunk carry into bx[:, n*TC]
        if cb > 0:
            cfix = trpool.tile([128, N], F32, tag="cfix")
            nc.vector.tensor_mul(cfix[:], dA3[:, :, 0],
                                 state[:, dt * N:(dt + 1) * N])
            nc.vector.tensor_add(bx3[:, :, 0], bx3[:, :, 0], cfix[:])
        # zero dA at every n-seam so one long scan resets per n
        # (h_seam = 0*prev + bx_seam; carry already folded into bx)
        nc.vector.memset(dA3[:, :, 0], 0.0)
        live[ch].setdefault("pre", {})[dt] = (dA, bx, bx3)

    def be_post(ch, dt):
        bb, cb = divmod(ch, CPB)
        h_t = live[ch]["h"][dt]
        dA, bx, bx3 = live[ch]["pre"].pop(dt)
        bcb_C3 = live[ch]["bcbC"][:].rearrange("p (n t) -> p n t", n=N)

        # the linear recurrence: one fused in-place scan (hs == bx)
        nc.vector.tensor_tensor_scan(
            bx[:, 0:N * TC], dA[:, 0:N * TC], bx[:, 0:N * TC],
            0.0, op0=ALU.mult, op1=ALU.add)
        if cb < CPB - 1:
            nc.gpsimd.tensor_copy(
                state[:, dt * N:(dt + 1) * N], bx3[:, :, TC - 1])

        # y_n = hs * C in place, then PE-reduce over n (+ D_skip * h)
        nc.vector.tensor_mul(bx3, bx3, bcb_C3)
        py = pp_y.tile([128, TC], F32, tag="py")
        for n in range(N):
            nc.tensor.matmul(py[:], dfr["eye"][:], bx[:, n * TC:(n + 1) * TC],
                             start=(n == 0), stop=False)
        nc.tensor.matmul(py[:], ddiag[dt][:], h_t[:],
                         start=False, stop=True)
        y_t = ypool.tile([128, TC], BF16, tag=f"y{dt}")
        nc.scalar.copy(y_t[:], py[:])
        live[ch].setdefault("y", [None] * NDT)[dt] = y_t

    def be_outproj(ch):
        bb, cb = divmod(ch, CPB)
        y_list = live[ch]["y"]
        for tt in range(TC // 128):
            po = pp_o.tile([128, DM], F32, tag="po")
            for dt in range(NDT):
                nc.tensor.matmul(
                    po[:], y_list[dt][:, tt * 128:(tt + 1) * 128],
                    out_wT[dt][:], start=(dt == 0), stop=False)
            nc.tensor.matmul(po[:], ones_bf[0:1, 0:128], dfr["b_out"][0:1, :],
                             start=False, stop=True)
            o_sb = opool.tile([128, DM], F32, tag="osb")
            nc.scalar.copy(o_sb[:], po[:])
            dma(outs["y_out"][bb, cb * TC + tt * 128:cb * TC + (tt + 1) * 128,
                              :], o_sb[:])
        live[ch].clear()

    # ---------------- pipelined emission ---------------------------------
    fe_prologue(0)
    for d in range(NDT):
        fe_dtile(0, d)
    load_deferred_consts()
    fe_stagec(0)
    be_pre(0, 0)
    for ch in range(NCH):
        nxt = ch + 1
        if nxt < NCH:
            fe_prologue(nxt)
        for d in range(NDT):
            # emit the NEXT d-tile's PE/ACT/bx work ahead of this scan
            if d + 1 < NDT:
                be_pre(ch, d + 1)
            elif nxt < NCH:
                be_pre(nxt, 0)
            be_post(ch, d)
            if nxt < NCH:
                for e in FE_SCHED.get(d, ()):
                    fe_dtile(nxt, e)
                if d == FE_STAGEC_AFTER:
                    fe_stagec(nxt)
        be_outproj(ch)

    st.close()


# ---------------------------------------------------------------- runner
_CACHE = {}


def _build_program():
    if "nc" in _CACHE:
        return _CACHE["nc"]
    nc = bacc.Bacc("TRN2", target_bir_lowering=False, debug=False,
                   num_devices=NCORES)
    ins, outs = declare_ios(nc)
    with tile.TileContext(nc) as t:
        emit(t, outs, ins)
    nc.compile()
    _CACHE["nc"] = nc
    return nc


LAST_RESULT = None


def kernel(**inputs) -> np.ndarray:
    global LAST_RESULT
    import os
    from concourse.bass_utils import run_bass_kernel_spmd

    nc = _build_program()
    w = host_weights(inputs)
    in_maps = []
    for c in range(NCORES):
        m = dict(w)
        m["xT"] = host_x_shard(inputs["x"], c)
        in_maps.append(m)
    trace = bool(os.environ.get("MIM_TRACE"))
    res = run_bass_kernel_spmd(nc, in_maps, list(range(NCORES)),
                               trace=trace)
    LAST_RESULT = res
    out = np.concatenate([res.results[c]["y_out"] for c in range(NCORES)],
                         axis=0)
    return out.astype(np.float32)


# revision 22
# speedup vs baseline: 1.2972x; 1.0026x over previous
"""Trainium2 Bass kernel for nn_MiM_v2 (Mamba-style selective scan).

Sharding: pure data-parallel over batch B=16 across 8 NeuronCores
(2 batches per core, weights replicated, no collectives).

v5: software-pipelined emission. Engine queues are in-order, so chunk
c+1's front end (in_proj/conv/silu/x_proj/rmsnorm/B,C broadcast) is
woven between chunk c's per-d-tile back end (dt_proj/softplus/deltaA/
scan/C-mult/PE reduce) to keep the Vector engine fed across chunk
boundaries. Scan runs in place (hs overwrites bx). The 16 per-n scans
are fused into one instruction per d-tile by zeroing dA at n-seams.
in/dt/x/out projections all bf16; n-reduction + D_skip on PE.
"""

import sys

if "/opt/trn_rl_repo" not in sys.path:
    sys.path.insert(0, "/opt/trn_rl_repo")

import numpy as np
import ml_dtypes

import concourse.bass as bass
import concourse.mybir as mybir
import concourse.tile as tile
from concourse import bacc

# ---------------------------------------------------------------- constants
B, L, DM = 16, 1024, 512
DIN, DT, N, K = 2 * DM, 32, 16, 3
NCORES = 8
BPC = B // NCORES          # batches per core
T = BPC * L                # tokens per core
TC = 512                   # token chunk
NCH = T // TC              # chunks per core
CPB = L // TC              # chunks per batch
NDT = DIN // 128           # d-inner tiles
NKT = DM // 128            # k tiles for in_proj
NSPL = 10                  # n-streams whose bx/C mults run on DVE (rest gpsimd)

F32 = mybir.dt.float32
F32R = mybir.dt.float32r
BF16 = mybir.dt.bfloat16
AF = mybir.ActivationFunctionType
ALU = mybir.AluOpType

# after back-end d-tile d of chunk c, emit these front-end d-tiles of c+1
FE_SCHED = {0: (0, 1), 1: (2,), 2: (3,), 3: (4,), 4: (5,), 5: (6, 7)}
FE_STAGEC_AFTER = 5


# ---------------------------------------------------------------- host prep
def host_weights(inp):
    """Precompute transposed/reorganized weights (numpy, shared by all cores)."""
    f = lambda x: np.ascontiguousarray(np.asarray(x, np.float32))
    bf = lambda x: np.ascontiguousarray(
        np.asarray(x, np.float32).astype(ml_dtypes.bfloat16))
    w = {}
    win_T = np.asarray(inp["in_w"], np.float32).T      # (DM, DIN)
    cw = np.asarray(inp["conv_w"], np.float32)[:, 0, :]  # (DIN, 3)
    b_in = np.asarray(inp["in_b"], np.float32)         # (DIN,)
    conv_b = np.asarray(inp["conv_b"], np.float32)     # (DIN,)
    # conv folded into in_proj: tap k scales column d of in_w.T
    w["w_in_aug"] = bf(np.stack([win_T * cw[None, :, k] for k in range(3)]))
    # folded bias: b_in * sum(w_k) + conv_b  (exact for t >= 2 and for
    # chunks with real left context)
    w["b_fold"] = bf((b_in * cw.sum(1) + conv_b)[None, :])
    # negative boundary correction for t=0,1 of each sequence, where the
    # causal zero-pad is in h-space (bias must not leak into the pad)
    ncorr = np.zeros((2, DIN), np.float32)
    ncorr[0] = -(cw[:, 0] + cw[:, 1]) * b_in
    ncorr[1] = -cw[:, 0] * b_in
    w["ncorr"] = bf(ncorr.reshape(1, 2 * DIN))
    onehot = np.zeros((2, TC), np.float32)
    onehot[0, 0] = 1.0
    onehot[1, 1] = 1.0
    w["onehot"] = bf(onehot.reshape(1, 2 * TC))
    w["w_x_T"] = bf(inp["xproj_w"].T)                  # (DIN, DT+2N) bf16
    w["w_dt_T"] = bf(inp["dt_w"].T)                    # (DT, DIN) bf16
    w["w_out_T"] = bf(inp["out_w"].T)                  # (DIN, DM) bf16
    w["A_neg"] = f(-np.exp(np.asarray(inp["A_log"], np.float64)))  # (DIN, N)
    w["b_dt"] = bf(inp["dt_b"][None, :])               # (1, DIN) bf16
    w["b_out"] = bf(inp["out_b"][None, :])             # (1, DM) bf16
    # block-diag D_skip pieces, one (128,128) diag per d-tile, bf16
    D = np.asarray(inp["D_skip"], np.float32)
    ddiag = np.zeros((NDT, 128, 128), np.float32)
    for k in range(NDT):
        ddiag[k] = np.diag(D[k * 128:(k + 1) * 128])
    w["ddiag"] = bf(ddiag)
    w["eye"] = bf(np.eye(128, dtype=np.float32))
    w["lnw"] = f(np.concatenate(
        [inp["dtln_w"], inp["Bln_w"], inp["Cln_w"]])[:, None])  # (64, 1)
    m_ms = np.zeros((DT + 2 * N, 3), np.float32)
    m_ms[:DT, 0] = 1.0 / DT
    m_ms[DT:DT + N, 1] = 1.0 / N
    m_ms[DT + N:, 2] = 1.0 / N
    w["m_ms"] = bf(m_ms)
    e_bc = np.zeros((3, DT + 2 * N), np.float32)
    e_bc[0, :DT] = 1.0
    e_bc[1, DT:DT + N] = 1.0
    e_bc[2, DT + N:] = 1.0
    w["e_bc"] = bf(e_bc)
    w["ones_bf"] = bf(np.ones((1, TC), np.float32))
    return w


def host_x_shard(x, core):
    """x (B, L, DM) -> per-core transposed bf16 shard (BPC, DM, L)."""
    xs = np.asarray(x, np.float32)[core * BPC:(core + 1) * BPC]
    return np.ascontiguousarray(
        xs.transpose(0, 2, 1).astype(ml_dtypes.bfloat16))


# ---------------------------------------------------------------- IO decl
def declare_ios(nc):
    def d(name, shape=None, dt=F32):
        return nc.dram_tensor(name, list(shape), dt,
                              kind="ExternalInput").ap()
    ins = {
        "xT": d("xT", dt=BF16, shape=(BPC, DM, L)),
        "w_in_aug": d("w_in_aug", dt=BF16, shape=(3, DM, DIN)),
        "b_fold": d("b_fold", dt=BF16, shape=(1, DIN)),
        "ncorr": d("ncorr", dt=BF16, shape=(1, 2 * DIN)),
        "onehot": d("onehot", dt=BF16, shape=(1, 2 * TC)),
        "w_x_T": d("w_x_T", dt=BF16, shape=(DIN, DT + 2 * N)),
        "w_dt_T": d("w_dt_T", dt=BF16, shape=(DT, DIN)),
        "w_out_T": d("w_out_T", dt=BF16, shape=(DIN, DM)),
        "A_neg": d("A_neg", (DIN, N)),
        "b_dt": d("b_dt", dt=BF16, shape=(1, DIN)),
        "b_out": d("b_out", dt=BF16, shape=(1, DM)),
        "ddiag": d("ddiag", dt=BF16, shape=(NDT, 128, 128)),
        "eye": d("eye", dt=BF16, shape=(128, 128)),
        "lnw": d("lnw", (DT + 2 * N, 1)),
        "m_ms": d("m_ms", dt=BF16, shape=(DT + 2 * N, 3)),
        "e_bc": d("e_bc", dt=BF16, shape=(3, DT + 2 * N)),
        "ones_bf": d("ones_bf", dt=BF16, shape=(1, TC)),
    }
    outs = {
        "y_out": nc.dram_tensor("y_out", [BPC, L, DM], F32,
                                kind="ExternalOutput").ap(),
    }
    return ins, outs


# ---------------------------------------------------------------- kernel body
def emit(tc_ctx, outs, ins):
    from contextlib import ExitStack
    tc = tc_ctx
    nc = tc.nc
    G = DT + 2 * N  # 64

    st = ExitStack()
    pool = lambda **kw: st.enter_context(tc.tile_pool(**kw))
    cpool = pool(name="consts", bufs=1)
    xpool = pool(name="xck", bufs=1)
    hpool = pool(name="h", bufs=2)
    trpool = pool(name="transient", bufs=2)
    spool = pool(name="smalls", bufs=1)
    dnpool = pool(name="dn", bufs=2)
    dApool = pool(name="dA", bufs=2)
    bxpool = pool(name="bx", bufs=2)
    bcpool = pool(name="bcb", bufs=2)
    bcpoolC = pool(name="bcc", bufs=1)
    ypool = pool(name="y", bufs=1)
    opool = pool(name="osb", bufs=1)
    pp_h = pool(name="ph", bufs=2, space="PSUM")
    pp_misc = pool(name="pmisc", bufs=2, space="PSUM")
    pp_y = pool(name="py", bufs=2, space="PSUM")
    pp_o = pool(name="po", bufs=2, space="PSUM")

    dma = nc.sync.dma_start

    # ---- persistent constants -------------------------------------------
    def const_tile(name, shape=None, src=None, dt=F32):
        t = cpool.tile(list(shape), dt, tag=name)
        if src.dtype != dt and mybir.dt.size(src.dtype) == mybir.dt.size(dt):
            src = src.bitcast(dt)
        dma(t[:], src)
        return t

    # critical consts for chunk-0 front end load first so the pipeline
    # starts immediately; the rest stream in behind the first in_projs.
    in_waug = [[const_tile(f"in_wA{k}_{kt}", (128, DIN),
                           ins["w_in_aug"][k, kt * 128:(kt + 1) * 128, :],
                           dt=BF16)
                for kt in range(NKT)] for k in range(3)]
    b_fold = const_tile("b_fold", dt=BF16, shape=(1, DIN),
                        src=ins["b_fold"][:, :])
    ncorr = const_tile("ncorr", (1, 2 * DIN), ins["ncorr"][:, :], dt=BF16)
    onehot = const_tile("onehot", (1, 2 * TC), ins["onehot"][:, :], dt=BF16)
    ones_bf = const_tile("ones_bf", (1, TC), ins["ones_bf"][:, :], dt=BF16)
    eps = cpool.tile([128, 1], F32, tag="eps")
    nc.vector.memset(eps[:], 1e-5)

    xproj_wT, out_wT, A_sb, ddiag = [], [], [], []
    dfr = {}

    def load_deferred_consts():
        xproj_wT.extend(const_tile(f"xp_wT{k}", (128, G),
                                   ins["w_x_T"][k * 128:(k + 1) * 128, :],
                                   dt=BF16) for k in range(NDT))
        dfr["dt_wT"] = const_tile("dt_wT", (DT, DIN), ins["w_dt_T"][:, :],
                                  dt=BF16)
        A_sb.extend(const_tile(f"A{k}", (128, N),
                               ins["A_neg"][k * 128:(k + 1) * 128, :])
                    for k in range(NDT))
        dfr["eye"] = const_tile("eye", (128, 128), ins["eye"][:, :], dt=BF16)
        dfr["b_dt"] = const_tile("b_dt", dt=BF16, shape=(1, DIN),
                                 src=ins["b_dt"][:, :])
        dfr["lnw"] = const_tile("lnw", (G, 1), ins["lnw"][:, :])
        dfr["m_ms"] = const_tile("m_ms", (G, 3), ins["m_ms"][:, :], dt=BF16)
        dfr["e_bc"] = const_tile("e_bc", (3, G), ins["e_bc"][:, :], dt=BF16)
        out_wT.extend(const_tile(f"out_wT{k}", (128, DM),
                                 ins["w_out_T"][k * 128:(k + 1) * 128, :],
                                 dt=BF16) for k in range(NDT))
        ddiag.extend(const_tile(f"dd{k}", (128, 128), ins["ddiag"][k],
                                dt=BF16) for k in range(NDT))
        dfr["b_out"] = const_tile("b_out", dt=BF16, shape=(1, DM),
                                  src=ins["b_out"][:, :])

    # persistent cross-chunk state
    state = cpool.tile([128, NDT * N], F32, tag="state")      # scan carries

    # DRAM bounce buffer for the B/C broadcast
    bc_dram = nc.dram_tensor("bc_scratch", [NCH, 2 * N, TC], BF16).ap()

    # per-chunk live objects for the pipelined emission
    live = [dict() for _ in range(NCH)]

    # ---------------- front end ------------------------------------------
    def fe_prologue(ch):
        bb, cb = divmod(ch, CPB)
        xck = []
        for kt in range(NKT):
            # 2 extra leading columns of left context for the fused conv
            t = xpool.tile([128, TC + 2], BF16, tag=f"x{kt}")
            if cb == 0:
                nc.vector.memset(t[:, 0:2], 0.0)
                dma(t[:, 2:TC + 2], ins["xT"][bb, kt * 128:(kt + 1) * 128,
                                              0:TC])
            else:
                dma(t[:], ins["xT"][bb, kt * 128:(kt + 1) * 128,
                                    cb * TC - 2:(cb + 1) * TC])
            xck.append(t)
        live[ch]["xck"] = xck
        live[ch]["h"] = [None] * NDT

    def fe_dtile(ch, dt):
        bb, cb = divmod(ch, CPB)
        xck = live[ch]["xck"]
        ph = pp_h.tile([128, TC], F32, tag="ph")
        # in_proj and causal conv fused: tap k uses x shifted by 2-k
        for k in range(3):
            for kt in range(NKT):
                nc.tensor.matmul(
                    ph[:], in_waug[k][kt][:, dt * 128:(dt + 1) * 128],
                    xck[kt][:, k:k + TC], start=(k == 0 and kt == 0),
                    stop=False)
        if cb == 0:
            # cancel the bias leaked into the h-space zero padding
            for r in range(2):
                nc.tensor.matmul(
                    ph[:],
                    ncorr[0:1, r * DIN + dt * 128:r * DIN + (dt + 1) * 128],
                    onehot[0:1, r * TC:(r + 1) * TC], start=False,
                    stop=False)
        nc.tensor.matmul(
            ph[:], b_fold[0:1, dt * 128:(dt + 1) * 128],
            ones_bf[0:1, 0:TC], start=False, stop=True)
        h_t = hpool.tile([128, TC], BF16, tag=f"h{dt}")
        nc.scalar.activation(h_t[:], ph[:], AF.Silu)
        live[ch]["h"][dt] = h_t

    def fe_stagec(ch):
        h_list = live[ch]["h"]
        pdbc = pp_misc.tile([G, TC], F32, tag="pmisc")
        for kt in range(NDT):
            nc.tensor.matmul(pdbc[:], xproj_wT[kt][:], h_list[kt][:],
                             start=(kt == 0), stop=(kt == NDT - 1))
        dbc_sb = spool.tile([G, TC], F32, tag="dbc")
        nc.scalar.copy(dbc_sb[:], pdbc[:])
        sq = spool.tile([G, TC], BF16, tag="sq")
        nc.scalar.activation(sq[:], pdbc[:], AF.Square)
        pms = pp_misc.tile([3, TC], F32, tag="pmisc")
        nc.tensor.matmul(pms[:], dfr["m_ms"][:], sq[:], start=True, stop=True)
        lnm = spool.tile([3, TC], F32, tag="lnm")
        nc.scalar.activation(lnm[:], pms[:], AF.Ln, bias=eps[0:3, :])
        rin = spool.tile([3, TC], BF16, tag="rin")
        nc.scalar.activation(rin[:], lnm[:], AF.Exp, scale=-0.5)
        pr = pp_misc.tile([G, TC], F32, tag="pmisc")
        nc.tensor.matmul(pr[:], dfr["e_bc"][:], rin[:], start=True, stop=True)
        delta_n = dnpool.tile([DT, TC], BF16, tag="dn")
        nc.vector.scalar_tensor_tensor(
            delta_n[:], dbc_sb[0:DT, :], dfr["lnw"][0:DT, :], pr[0:DT, :],
            op0=ALU.mult, op1=ALU.mult)
        bc_n = spool.tile([2 * N, TC], BF16, tag="bcn")
        nc.vector.scalar_tensor_tensor(
            bc_n[:], dbc_sb[DT:G, :], dfr["lnw"][DT:G, :], pr[DT:G, :],
            op0=ALU.mult, op1=ALU.mult)

        # bounce B/C rows through DRAM to broadcast across 128 partitions
        dma(bc_dram[ch], bc_n[:])
        bcbB = bcpool.tile([128, N * TC], BF16, tag="bcb")
        nc.sync.dma_start(
            bcbB[:].rearrange("p (j t) -> p j t", j=N),
            bc_dram[ch, 0:N].unsqueeze(0).broadcast_to((128, N, TC)))
        bcbC = bcpoolC.tile([128, N * TC], BF16, tag="bcc")
        nc.sync.dma_start(
            bcbC[:].rearrange("p (j t) -> p j t", j=N),
            bc_dram[ch, N:2 * N].unsqueeze(0).broadcast_to((128, N, TC)))
        live[ch]["bcbB"] = bcbB
        live[ch]["bcbC"] = bcbC
        live[ch]["dn"] = delta_n

    # ---------------- back end -------------------------------------------
    def be_pre(ch, dt):
        """dt_proj -> softplus -> u, deltaA exps, bx build + carry fixup.
        Emitted one d-tile ahead so PE/ACT results are ready when the
        Vector engine reaches this d-tile's scan."""
        bb, cb = divmod(ch, CPB)
        h_t = live[ch]["h"][dt]
        delta_n = live[ch]["dn"]
        bcb_B3 = live[ch]["bcbB"][:].rearrange("p (n t) -> p n t", n=N)

        pd = pp_h.tile([128, TC], F32, tag="ph")
        nc.tensor.matmul(pd[:], dfr["dt_wT"][:, dt * 128:(dt + 1) * 128],
                         delta_n[:], start=True, stop=False)
        nc.tensor.matmul(pd[:], dfr["b_dt"][0:1, dt * 128:(dt + 1) * 128],
                         ones_bf[0:1, 0:TC], start=False, stop=True)
        esp = trpool.tile([128, TC], BF16, tag="esp")
        nc.scalar.activation(esp[:], pd[:], AF.Exp)
        delta_t = trpool.tile([128, TC], BF16, tag="delta")
        nc.scalar.activation(delta_t[:], esp[:], AF.Ln, bias=1.0)
        u_t = trpool.tile([128, TC], BF16, tag="u")
        nc.vector.tensor_mul(u_t[:], delta_t[:], h_t[:])

        # deltaA = exp(A_n * delta), bf16, one [128, N*TC] tile
        dA = dApool.tile([128, N * TC], BF16, tag="dA")
        for n in range(N):
            nc.scalar.activation(
                dA[:, n * TC:(n + 1) * TC], delta_t[:], AF.Exp,
                scale=A_sb[dt][:, n:n + 1])

        # bx = u * B (broadcast u over n), split between gpsimd and DVE
        bx = bxpool.tile([128, N * TC], BF16, tag="bx")
        bx3 = bx[:].rearrange("p (n t) -> p n t", n=N)
        u3 = u_t[:].unsqueeze(1).broadcast_to((128, N, TC))
        nc.vector.tensor_mul(bx3, u3, bcb_B3)

        dA3 = dA[:].rearrange("p (n t) -> p n t", n=N)
        # fold cross-chunk carry into bx[:, n*TC]
        if cb > 0:
            cfix = trpool.tile([128, N], F32, tag="cfix")
            nc.vector.tensor_mul(cfix[:], dA3[:, :, 0],
                                 state[:, dt * N:(dt + 1) * N])
            nc.vector.tensor_add(bx3[:, :, 0], bx3[:, :, 0], cfix[:])
        # zero dA at every n-seam so one long scan resets per n
        # (h_seam = 0*prev + bx_seam; carry already folded into bx)
        nc.vector.memset(dA3[:, :, 0], 0.0)
        live[ch].setdefault("pre", {})[dt] = (dA, bx, bx3)

    def be_post(ch, dt):
        bb, cb = divmod(ch, CPB)
        h_t = live[ch]["h"][dt]
        dA, bx, bx3 = live[ch]["pre"].pop(dt)
        bcb_C3 = live[ch]["bcbC"][:].rearrange("p (n t) -> p n t", n=N)

        # the linear recurrence: one fused in-place scan (hs == bx)
        nc.vector.tensor_tensor_scan(
            bx[:, 0:N * TC], dA[:, 0:N * TC], bx[:, 0:N * TC],
            0.0, op0=ALU.mult, op1=ALU.add)
        if cb < CPB - 1:
            nc.vector.tensor_copy(
                state[:, dt * N:(dt + 1) * N], bx3[:, :, TC - 1])

        # y_n = hs * C in place, then PE-reduce over n (+ D_skip * h)
        nc.vector.tensor_mul(bx3, bx3, bcb_C3)
        py = pp_y.tile([128, TC], F32, tag="py")
        for n in range(N):
            nc.tensor.matmul(py[:], dfr["eye"][:], bx[:, n * TC:(n + 1) * TC],
                             start=(n == 0), stop=False)
        nc.tensor.matmul(py[:], ddiag[dt][:], h_t[:],
                         start=False, stop=True)
        y_t = ypool.tile([128, TC], BF16, tag=f"y{dt}")
        nc.scalar.copy(y_t[:], py[:])
        live[ch].setdefault("y", [None] * NDT)[dt] = y_t

    def be_outproj(ch):
        bb, cb = divmod(ch, CPB)
        y_list = live[ch]["y"]
        for tt in range(TC // 128):
            po = pp_o.tile([128, DM], F32, tag="po")
            for dt in range(NDT):
                nc.tensor.matmul(
                    po[:], y_list[dt][:, tt * 128:(tt + 1) * 128],
                    out_wT[dt][:], start=(dt == 0), stop=False)
            nc.tensor.matmul(po[:], ones_bf[0:1, 0:128], dfr["b_out"][0:1, :],
                             start=False, stop=True)
            o_sb = opool.tile([128, DM], F32, tag="osb")
            nc.scalar.copy(o_sb[:], po[:])
            dma(outs["y_out"][bb, cb * TC + tt * 128:cb * TC + (tt + 1) * 128,
                              :], o_sb[:])
        live[ch].clear()

    # ---------------- pipelined emission ---------------------------------
    fe_prologue(0)
    for d in range(NDT):
        fe_dtile(0, d)
    load_deferred_consts()
    fe_stagec(0)
    be_pre(0, 0)
    for ch in range(NCH):
        nxt = ch + 1
        if nxt < NCH:
            fe_prologue(nxt)
        for d in range(NDT):
            # emit the NEXT d-tile's PE/ACT/bx work ahead of this scan
            if d + 1 < NDT:
                be_pre(ch, d + 1)
            elif nxt < NCH:
                be_pre(nxt, 0)
            be_post(ch, d)
            if nxt < NCH:
                for e in FE_SCHED.get(d, ()):
                    fe_dtile(nxt, e)
                if d == FE_STAGEC_AFTER:
                    fe_stagec(nxt)
        be_outproj(ch)

    st.close()


# ---------------------------------------------------------------- runner
_CACHE = {}


def _build_program():
    if "nc" in _CACHE:
        return _CACHE["nc"]
    nc = bacc.Bacc("TRN2", target_bir_lowering=False, debug=False,
                   num_devices=NCORES)
    ins, outs = declare_ios(nc)
    with tile.TileContext(nc) as t:
        emit(t, outs, ins)
    nc.compile()
    _CACHE["nc"] = nc
    return nc


LAST_RESULT = None


def kernel(**inputs) -> np.ndarray:
    global LAST_RESULT
    import os
    from concourse.bass_utils import run_bass_kernel_spmd

    nc = _build_program()
    w = host_weights(inputs)
    in_maps = []
    for c in range(NCORES):
        m = dict(w)
        m["xT"] = host_x_shard(inputs["x"], c)
        in_maps.append(m)
    trace = bool(os.environ.get("MIM_TRACE"))
    res = run_bass_kernel_spmd(nc, in_maps, list(range(NCORES)),
                               trace=trace)
    LAST_RESULT = res
    out = np.concatenate([res.results[c]["y_out"] for c in range(NCORES)],
                         axis=0)
    return out.astype(np.float32)


# revision 24
# speedup vs baseline: 1.3013x; 1.0031x over previous
"""Trainium2 Bass kernel for nn_MiM_v2 (Mamba-style selective scan).

Sharding: pure data-parallel over batch B=16 across 8 NeuronCores
(2 batches per core, weights replicated, no collectives).

v5: software-pipelined emission. Engine queues are in-order, so chunk
c+1's front end (in_proj/conv/silu/x_proj/rmsnorm/B,C broadcast) is
woven between chunk c's per-d-tile back end (dt_proj/softplus/deltaA/
scan/C-mult/PE reduce) to keep the Vector engine fed across chunk
boundaries. Scan runs in place (hs overwrites bx). The 16 per-n scans
are fused into one instruction per d-tile by zeroing dA at n-seams.
in/dt/x/out projections all bf16; n-reduction + D_skip on PE.
"""

import sys

if "/opt/trn_rl_repo" not in sys.path:
    sys.path.insert(0, "/opt/trn_rl_repo")

import numpy as np
import ml_dtypes

import concourse.bass as bass
import concourse.mybir as mybir
import concourse.tile as tile
from concourse import bacc

# ---------------------------------------------------------------- constants
B, L, DM = 16, 1024, 512
DIN, DT, N, K = 2 * DM, 32, 16, 3
NCORES = 8
BPC = B // NCORES          # batches per core
T = BPC * L                # tokens per core
TC = 512                   # token chunk
NCH = T // TC              # chunks per core
CPB = L // TC              # chunks per batch
NDT = DIN // 128           # d-inner tiles
NKT = DM // 128            # k tiles for in_proj
NSPL = 10                  # n-streams whose bx/C mults run on DVE (rest gpsimd)

F32 = mybir.dt.float32
F32R = mybir.dt.float32r
BF16 = mybir.dt.bfloat16
AF = mybir.ActivationFunctionType
ALU = mybir.AluOpType

# after back-end d-tile d of chunk c, emit these front-end d-tiles of c+1
FE_SCHED = {0: (0, 1), 1: (2, 3), 2: (4, 5), 3: (6, 7)}
FE_STAGEC_AFTER = 4


# ---------------------------------------------------------------- host prep
def host_weights(inp):
    """Precompute transposed/reorganized weights (numpy, shared by all cores)."""
    f = lambda x: np.ascontiguousarray(np.asarray(x, np.float32))
    bf = lambda x: np.ascontiguousarray(
        np.asarray(x, np.float32).astype(ml_dtypes.bfloat16))
    w = {}
    win_T = np.asarray(inp["in_w"], np.float32).T      # (DM, DIN)
    cw = np.asarray(inp["conv_w"], np.float32)[:, 0, :]  # (DIN, 3)
    b_in = np.asarray(inp["in_b"], np.float32)         # (DIN,)
    conv_b = np.asarray(inp["conv_b"], np.float32)     # (DIN,)
    # conv folded into in_proj: tap k scales column d of in_w.T
    w["w_in_aug"] = bf(np.stack([win_T * cw[None, :, k] for k in range(3)]))
    # folded bias: b_in * sum(w_k) + conv_b  (exact for t >= 2 and for
    # chunks with real left context)
    w["b_fold"] = bf((b_in * cw.sum(1) + conv_b)[None, :])
    # negative boundary correction for t=0,1 of each sequence, where the
    # causal zero-pad is in h-space (bias must not leak into the pad)
    ncorr = np.zeros((2, DIN), np.float32)
    ncorr[0] = -(cw[:, 0] + cw[:, 1]) * b_in
    ncorr[1] = -cw[:, 0] * b_in
    w["ncorr"] = bf(ncorr.reshape(1, 2 * DIN))
    onehot = np.zeros((2, TC), np.float32)
    onehot[0, 0] = 1.0
    onehot[1, 1] = 1.0
    w["onehot"] = bf(onehot.reshape(1, 2 * TC))
    w["w_x_T"] = bf(inp["xproj_w"].T)                  # (DIN, DT+2N) bf16
    w["w_dt_T"] = bf(inp["dt_w"].T)                    # (DT, DIN) bf16
    w["w_out_T"] = bf(inp["out_w"].T)                  # (DIN, DM) bf16
    w["A_neg"] = f(-np.exp(np.asarray(inp["A_log"], np.float64)))  # (DIN, N)
    w["b_dt"] = bf(inp["dt_b"][None, :])               # (1, DIN) bf16
    w["b_out"] = bf(inp["out_b"][None, :])             # (1, DM) bf16
    # block-diag D_skip pieces, one (128,128) diag per d-tile, bf16
    D = np.asarray(inp["D_skip"], np.float32)
    ddiag = np.zeros((NDT, 128, 128), np.float32)
    for k in range(NDT):
        ddiag[k] = np.diag(D[k * 128:(k + 1) * 128])
    w["ddiag"] = bf(ddiag)
    w["eye"] = bf(np.eye(128, dtype=np.float32))
    w["lnw"] = f(np.concatenate(
        [inp["dtln_w"], inp["Bln_w"], inp["Cln_w"]])[:, None])  # (64, 1)
    m_ms = np.zeros((DT + 2 * N, 3), np.float32)
    m_ms[:DT, 0] = 1.0 / DT
    m_ms[DT:DT + N, 1] = 1.0 / N
    m_ms[DT + N:, 2] = 1.0 / N
    w["m_ms"] = bf(m_ms)
    e_bc = np.zeros((3, DT + 2 * N), np.float32)
    e_bc[0, :DT] = 1.0
    e_bc[1, DT:DT + N] = 1.0
    e_bc[2, DT + N:] = 1.0
    w["e_bc"] = bf(e_bc)
    w["ones_bf"] = bf(np.ones((1, TC), np.float32))
    return w


def host_x_shard(x, core):
    """x (B, L, DM) -> per-core transposed bf16 shard (BPC, DM, L)."""
    xs = np.asarray(x, np.float32)[core * BPC:(core + 1) * BPC]
    return np.ascontiguousarray(
        xs.transpose(0, 2, 1).astype(ml_dtypes.bfloat16))


# ---------------------------------------------------------------- IO decl
def declare_ios(nc):
    def d(name, shape=None, dt=F32):
        return nc.dram_tensor(name, list(shape), dt,
                              kind="ExternalInput").ap()
    ins = {
        "xT": d("xT", dt=BF16, shape=(BPC, DM, L)),
        "w_in_aug": d("w_in_aug", dt=BF16, shape=(3, DM, DIN)),
        "b_fold": d("b_fold", dt=BF16, shape=(1, DIN)),
        "ncorr": d("ncorr", dt=BF16, shape=(1, 2 * DIN)),
        "onehot": d("onehot", dt=BF16, shape=(1, 2 * TC)),
        "w_x_T": d("w_x_T", dt=BF16, shape=(DIN, DT + 2 * N)),
        "w_dt_T": d("w_dt_T", dt=BF16, shape=(DT, DIN)),
        "w_out_T": d("w_out_T", dt=BF16, shape=(DIN, DM)),
        "A_neg": d("A_neg", (DIN, N)),
        "b_dt": d("b_dt", dt=BF16, shape=(1, DIN)),
        "b_out": d("b_out", dt=BF16, shape=(1, DM)),
        "ddiag": d("ddiag", dt=BF16, shape=(NDT, 128, 128)),
        "eye": d("eye", dt=BF16, shape=(128, 128)),
        "lnw": d("lnw", (DT + 2 * N, 1)),
        "m_ms": d("m_ms", dt=BF16, shape=(DT + 2 * N, 3)),
        "e_bc": d("e_bc", dt=BF16, shape=(3, DT + 2 * N)),
        "ones_bf": d("ones_bf", dt=BF16, shape=(1, TC)),
    }
    outs = {
        "y_out": nc.dram_tensor("y_out", [BPC, L, DM], F32,
                                kind="ExternalOutput").ap(),
    }
    return ins, outs


# ---------------------------------------------------------------- kernel body
def emit(tc_ctx, outs, ins):
    from contextlib import ExitStack
    tc = tc_ctx
    nc = tc.nc
    G = DT + 2 * N  # 64

    st = ExitStack()
    pool = lambda **kw: st.enter_context(tc.tile_pool(**kw))
    cpool = pool(name="consts", bufs=1)
    xpool = pool(name="xck", bufs=1)
    hpool = pool(name="h", bufs=2)
    trpool = pool(name="transient", bufs=2)
    spool = pool(name="smalls", bufs=1)
    dnpool = pool(name="dn", bufs=2)
    dApool = pool(name="dA", bufs=2)
    bxpool = pool(name="bx", bufs=2)
    bcpool = pool(name="bcb", bufs=2)
    bcpoolC = pool(name="bcc", bufs=1)
    ypool = pool(name="y", bufs=1)
    opool = pool(name="osb", bufs=1)
    pp_h = pool(name="ph", bufs=2, space="PSUM")
    pp_misc = pool(name="pmisc", bufs=2, space="PSUM")
    pp_y = pool(name="py", bufs=2, space="PSUM")
    pp_o = pool(name="po", bufs=2, space="PSUM")

    dma = nc.sync.dma_start

    # ---- persistent constants -------------------------------------------
    def const_tile(name, shape=None, src=None, dt=F32):
        t = cpool.tile(list(shape), dt, tag=name)
        if src.dtype != dt and mybir.dt.size(src.dtype) == mybir.dt.size(dt):
            src = src.bitcast(dt)
        dma(t[:], src)
        return t

    # critical consts for chunk-0 front end load first so the pipeline
    # starts immediately; the rest stream in behind the first in_projs.
    in_waug = [[const_tile(f"in_wA{k}_{kt}", (128, DIN),
                           ins["w_in_aug"][k, kt * 128:(kt + 1) * 128, :],
                           dt=BF16)
                for kt in range(NKT)] for k in range(3)]
    b_fold = const_tile("b_fold", dt=BF16, shape=(1, DIN),
                        src=ins["b_fold"][:, :])
    ncorr = const_tile("ncorr", (1, 2 * DIN), ins["ncorr"][:, :], dt=BF16)
    onehot = const_tile("onehot", (1, 2 * TC), ins["onehot"][:, :], dt=BF16)
    ones_bf = const_tile("ones_bf", (1, TC), ins["ones_bf"][:, :], dt=BF16)
    eps = cpool.tile([128, 1], F32, tag="eps")
    nc.vector.memset(eps[:], 1e-5)

    xproj_wT, out_wT, A_sb, ddiag = [], [], [], []
    dfr = {}

    def load_deferred_consts():
        xproj_wT.extend(const_tile(f"xp_wT{k}", (128, G),
                                   ins["w_x_T"][k * 128:(k + 1) * 128, :],
                                   dt=BF16) for k in range(NDT))
        dfr["dt_wT"] = const_tile("dt_wT", (DT, DIN), ins["w_dt_T"][:, :],
                                  dt=BF16)
        A_sb.extend(const_tile(f"A{k}", (128, N),
                               ins["A_neg"][k * 128:(k + 1) * 128, :])
                    for k in range(NDT))
        dfr["eye"] = const_tile("eye", (128, 128), ins["eye"][:, :], dt=BF16)
        dfr["b_dt"] = const_tile("b_dt", dt=BF16, shape=(1, DIN),
                                 src=ins["b_dt"][:, :])
        dfr["lnw"] = const_tile("lnw", (G, 1), ins["lnw"][:, :])
        dfr["m_ms"] = const_tile("m_ms", (G, 3), ins["m_ms"][:, :], dt=BF16)
        dfr["e_bc"] = const_tile("e_bc", (3, G), ins["e_bc"][:, :], dt=BF16)
        out_wT.extend(const_tile(f"out_wT{k}", (128, DM),
                                 ins["w_out_T"][k * 128:(k + 1) * 128, :],
                                 dt=BF16) for k in range(NDT))
        ddiag.extend(const_tile(f"dd{k}", (128, 128), ins["ddiag"][k],
                                dt=BF16) for k in range(NDT))
        dfr["b_out"] = const_tile("b_out", dt=BF16, shape=(1, DM),
                                  src=ins["b_out"][:, :])

    # persistent cross-chunk state
    state = cpool.tile([128, NDT * N], F32, tag="state")      # scan carries

    # DRAM bounce buffer for the B/C broadcast
    bc_dram = nc.dram_tensor("bc_scratch", [NCH, 2 * N, TC], BF16).ap()

    # per-chunk live objects for the pipelined emission
    live = [dict() for _ in range(NCH)]

    # ---------------- front end ------------------------------------------
    def fe_prologue(ch):
        bb, cb = divmod(ch, CPB)
        xck = []
        for kt in range(NKT):
            # 2 extra leading columns of left context for the fused conv
            t = xpool.tile([128, TC + 2], BF16, tag=f"x{kt}")
            if cb == 0:
                nc.vector.memset(t[:, 0:2], 0.0)
                dma(t[:, 2:TC + 2], ins["xT"][bb, kt * 128:(kt + 1) * 128,
                                              0:TC])
            else:
                dma(t[:], ins["xT"][bb, kt * 128:(kt + 1) * 128,
                                    cb * TC - 2:(cb + 1) * TC])
            xck.append(t)
        live[ch]["xck"] = xck
        live[ch]["h"] = [None] * NDT

    def fe_dtile(ch, dt):
        bb, cb = divmod(ch, CPB)
        xck = live[ch]["xck"]
        ph = pp_h.tile([128, TC], F32, tag="ph")
        # in_proj and causal conv fused: tap k uses x shifted by 2-k
        for k in range(3):
            for kt in range(NKT):
                nc.tensor.matmul(
                    ph[:], in_waug[k][kt][:, dt * 128:(dt + 1) * 128],
                    xck[kt][:, k:k + TC], start=(k == 0 and kt == 0),
                    stop=False)
        if cb == 0:
            # cancel the bias leaked into the h-space zero padding
            for r in range(2):
                nc.tensor.matmul(
                    ph[:],
                    ncorr[0:1, r * DIN + dt * 128:r * DIN + (dt + 1) * 128],
                    onehot[0:1, r * TC:(r + 1) * TC], start=False,
                    stop=False)
        nc.tensor.matmul(
            ph[:], b_fold[0:1, dt * 128:(dt + 1) * 128],
            ones_bf[0:1, 0:TC], start=False, stop=True)
        h_t = hpool.tile([128, TC], BF16, tag=f"h{dt}")
        nc.scalar.activation(h_t[:], ph[:], AF.Silu)
        live[ch]["h"][dt] = h_t

    def fe_stagec(ch):
        h_list = live[ch]["h"]
        pdbc = pp_misc.tile([G, TC], F32, tag="pmisc")
        for kt in range(NDT):
            nc.tensor.matmul(pdbc[:], xproj_wT[kt][:], h_list[kt][:],
                             start=(kt == 0), stop=(kt == NDT - 1))
        dbc_sb = spool.tile([G, TC], F32, tag="dbc")
        nc.scalar.copy(dbc_sb[:], pdbc[:])
        sq = spool.tile([G, TC], BF16, tag="sq")
        nc.scalar.activation(sq[:], pdbc[:], AF.Square)
        pms = pp_misc.tile([3, TC], F32, tag="pmisc")
        nc.tensor.matmul(pms[:], dfr["m_ms"][:], sq[:], start=True, stop=True)
        lnm = spool.tile([3, TC], F32, tag="lnm")
        nc.scalar.activation(lnm[:], pms[:], AF.Ln, bias=eps[0:3, :])
        rin = spool.tile([3, TC], BF16, tag="rin")
        nc.scalar.activation(rin[:], lnm[:], AF.Exp, scale=-0.5)
        pr = pp_misc.tile([G, TC], F32, tag="pmisc")
        nc.tensor.matmul(pr[:], dfr["e_bc"][:], rin[:], start=True, stop=True)
        delta_n = dnpool.tile([DT, TC], BF16, tag="dn")
        nc.vector.scalar_tensor_tensor(
            delta_n[:], dbc_sb[0:DT, :], dfr["lnw"][0:DT, :], pr[0:DT, :],
            op0=ALU.mult, op1=ALU.mult)
        bc_n = spool.tile([2 * N, TC], BF16, tag="bcn")
        nc.vector.scalar_tensor_tensor(
            bc_n[:], dbc_sb[DT:G, :], dfr["lnw"][DT:G, :], pr[DT:G, :],
            op0=ALU.mult, op1=ALU.mult)

        # bounce B/C rows through DRAM to broadcast across 128 partitions
        dma(bc_dram[ch], bc_n[:])
        bcbB = bcpool.tile([128, N * TC], BF16, tag="bcb")
        nc.sync.dma_start(
            bcbB[:].rearrange("p (j t) -> p j t", j=N),
            bc_dram[ch, 0:N].unsqueeze(0).broadcast_to((128, N, TC)))
        bcbC = bcpoolC.tile([128, N * TC], BF16, tag="bcc")
        nc.sync.dma_start(
            bcbC[:].rearrange("p (j t) -> p j t", j=N),
            bc_dram[ch, N:2 * N].unsqueeze(0).broadcast_to((128, N, TC)))
        live[ch]["bcbB"] = bcbB
        live[ch]["bcbC"] = bcbC
        live[ch]["dn"] = delta_n

    # ---------------- back end -------------------------------------------
    def be_pre(ch, dt):
        """dt_proj -> softplus -> u, deltaA exps, bx build + carry fixup.
        Emitted one d-tile ahead so PE/ACT results are ready when the
        Vector engine reaches this d-tile's scan."""
        bb, cb = divmod(ch, CPB)
        h_t = live[ch]["h"][dt]
        delta_n = live[ch]["dn"]
        bcb_B3 = live[ch]["bcbB"][:].rearrange("p (n t) -> p n t", n=N)

        pd = pp_h.tile([128, TC], F32, tag="ph")
        nc.tensor.matmul(pd[:], dfr["dt_wT"][:, dt * 128:(dt + 1) * 128],
                         delta_n[:], start=True, stop=False)
        nc.tensor.matmul(pd[:], dfr["b_dt"][0:1, dt * 128:(dt + 1) * 128],
                         ones_bf[0:1, 0:TC], start=False, stop=True)
        esp = trpool.tile([128, TC], BF16, tag="esp")
        nc.scalar.activation(esp[:], pd[:], AF.Exp)
        delta_t = trpool.tile([128, TC], BF16, tag="delta")
        nc.scalar.activation(delta_t[:], esp[:], AF.Ln, bias=1.0)
        u_t = trpool.tile([128, TC], BF16, tag="u")
        nc.vector.tensor_mul(u_t[:], delta_t[:], h_t[:])

        # deltaA = exp(A_n * delta), bf16, one [128, N*TC] tile
        dA = dApool.tile([128, N * TC], BF16, tag="dA")
        for n in range(N):
            nc.scalar.activation(
                dA[:, n * TC:(n + 1) * TC], delta_t[:], AF.Exp,
                scale=A_sb[dt][:, n:n + 1])

        # bx = u * B (broadcast u over n), split between gpsimd and DVE
        bx = bxpool.tile([128, N * TC], BF16, tag="bx")
        bx3 = bx[:].rearrange("p (n t) -> p n t", n=N)
        u3 = u_t[:].unsqueeze(1).broadcast_to((128, N, TC))
        nc.vector.tensor_mul(bx3, u3, bcb_B3)

        dA3 = dA[:].rearrange("p (n t) -> p n t", n=N)
        # fold cross-chunk carry into bx[:, n*TC]
        if cb > 0:
            cfix = trpool.tile([128, N], F32, tag="cfix")
            nc.vector.tensor_mul(cfix[:], dA3[:, :, 0],
                                 state[:, dt * N:(dt + 1) * N])
            nc.vector.tensor_add(bx3[:, :, 0], bx3[:, :, 0], cfix[:])
        # zero dA at every n-seam so one long scan resets per n
        # (h_seam = 0*prev + bx_seam; carry already folded into bx)
        nc.vector.memset(dA3[:, :, 0], 0.0)
        live[ch].setdefault("pre", {})[dt] = (dA, bx, bx3)

    def be_post(ch, dt):
        bb, cb = divmod(ch, CPB)
        h_t = live[ch]["h"][dt]
        dA, bx, bx3 = live[ch]["pre"].pop(dt)
        bcb_C3 = live[ch]["bcbC"][:].rearrange("p (n t) -> p n t", n=N)

        # the linear recurrence: one fused in-place scan (hs == bx)
        nc.vector.tensor_tensor_scan(
            bx[:, 0:N * TC], dA[:, 0:N * TC], bx[:, 0:N * TC],
            0.0, op0=ALU.mult, op1=ALU.add)
        if cb < CPB - 1:
            nc.vector.tensor_copy(
                state[:, dt * N:(dt + 1) * N], bx3[:, :, TC - 1])

        # y_n = hs * C in place, then PE-reduce over n (+ D_skip * h)
        nc.vector.tensor_mul(bx3, bx3, bcb_C3)
        py = pp_y.tile([128, TC], F32, tag="py")
        for n in range(N):
            nc.tensor.matmul(py[:], dfr["eye"][:], bx[:, n * TC:(n + 1) * TC],
                             start=(n == 0), stop=False)
        nc.tensor.matmul(py[:], ddiag[dt][:], h_t[:],
                         start=False, stop=True)
        y_t = ypool.tile([128, TC], BF16, tag=f"y{dt}")
        nc.scalar.copy(y_t[:], py[:])
        live[ch].setdefault("y", [None] * NDT)[dt] = y_t

    def be_outproj(ch):
        bb, cb = divmod(ch, CPB)
        y_list = live[ch]["y"]
        for tt in range(TC // 128):
            po = pp_o.tile([128, DM], F32, tag="po")
            for dt in range(NDT):
                nc.tensor.matmul(
                    po[:], y_list[dt][:, tt * 128:(tt + 1) * 128],
                    out_wT[dt][:], start=(dt == 0), stop=False)
            nc.tensor.matmul(po[:], ones_bf[0:1, 0:128], dfr["b_out"][0:1, :],
                             start=False, stop=True)
            o_sb = opool.tile([128, DM], F32, tag="osb")
            nc.scalar.copy(o_sb[:], po[:])
            dma(outs["y_out"][bb, cb * TC + tt * 128:cb * TC + (tt + 1) * 128,
                              :], o_sb[:])
        live[ch].clear()

    # ---------------- pipelined emission ---------------------------------
    fe_prologue(0)
    for d in range(NDT):
        fe_dtile(0, d)
    load_deferred_consts()
    fe_stagec(0)
    be_pre(0, 0)
    for ch in range(NCH):
        nxt = ch + 1
        if nxt < NCH:
            fe_prologue(nxt)
        for d in range(NDT):
            # emit the NEXT d-tile's PE/ACT/bx work ahead of this scan
            if d + 1 < NDT:
                be_pre(ch, d + 1)
            elif nxt < NCH:
                be_pre(nxt, 0)
            be_post(ch, d)
            if nxt < NCH:
                for e in FE_SCHED.get(d, ()):
                    fe_dtile(nxt, e)
                if d == FE_STAGEC_AFTER:
                    fe_stagec(nxt)
        be_outproj(ch)

    st.close()


# ---------------------------------------------------------------- runner
_CACHE = {}


def _build_program():
    if "nc" in _CACHE:
        return _CACHE["nc"]
    nc = bacc.Bacc("TRN2", target_bir_lowering=False, debug=False,
                   num_devices=NCORES)
    ins, outs = declare_ios(nc)
    with tile.TileContext(nc) as t:
        emit(t, outs, ins)
    nc.compile()
    _CACHE["nc"] = nc
    return nc


LAST_RESULT = None


def kernel(**inputs) -> np.ndarray:
    global LAST_RESULT
    import os
    from concourse.bass_utils import run_bass_kernel_spmd

    nc = _build_program()
    w = host_weights(inputs)
    in_maps = []
    for c in range(NCORES):
        m = dict(w)
        m["xT"] = host_x_shard(inputs["x"], c)
        in_maps.append(m)
    trace = bool(os.environ.get("MIM_TRACE"))
    res = run_bass_kernel_spmd(nc, in_maps, list(range(NCORES)),
                               trace=trace)
    LAST_RESULT = res
    out = np.concatenate([res.results[c]["y_out"] for c in range(NCORES)],
                         axis=0)
    return out.astype(np.float32)
